# revision 27
# baseline (speedup 1.0000x reference)
"""Masked dot-product attention on 8 Trainium2 NeuronCores.

Full inputs: queries/keys/values [8, 2048, 128] f32, valid_lens [8] i32.
Output: softmax(Q K^T / sqrt(128), masked to valid_lens) @ V, [8, 2048, 128] f32.

Strategy
--------
Keys at positions >= valid_lens[b] carry zero softmax weight, so only
ceil(vl[b]/128) key-chunks per batch matter.  Scores are O(6), so softmax
needs no max-subtraction and partial (numerator, denominator) sums over
disjoint key ranges are additive -- work splits across cores and is
recombined on the host.  Masking is applied entirely on the V side: the
host zeroes V rows and the ones-column for invalid keys, so their
(finite, garbage) exp weights contribute exactly 0 to both numerator and
denominator and the exponentials need no masking at all.

The device program is a flat software pipeline over "half-chunks"
(128 keys x 512 queries).  Half-chunks are grouped into slots of 512
queries; slots come in PAIRS covering the two query-halves of one
(batch, key-range) segment, sharing one K^T/V input block (halves input
DMA).  Adjacent chunks form exp UNITS of two (one odd single per slot)
so the fixed per-instruction PSUM-access cost of the exponent engines is
paid once per 2 chunks.  Per unit:
  S^T   = K_chunk @ Q^T               2 matmuls -> one [128k x 1024q]
                                      2-bank PSUM tile
  P^T   = exp(SCALE*S^T)              ScalarE native exp on 896 cols,
          VectorE Schraudolph fast-exp on 128 cols (one
          scalar_tensor_tensor producing bf16 bit patterns in int16)
  PV   += P^T_j^T @ [V_chunk | 1]     4 matmuls (129 cols) per chunk,
          accumulating with start=False onto pre-zeroed PSUM ranges
          (matmul start=True resets a whole bank, so packed groups
          cannot use it); ones-column = softmax denominator
pv bank A (per slot parity) holds j0-2, bank B is shared (j3 of even /
odd slots at different columns): 3 pv banks + 4 st banks.  At slot end
the two pv ranges are copied (Vector + Scalar) to a bf16 stage tile,
DMA'd partition-major (contiguous 1032B rows), and the banks re-zeroed
for the slot two ahead -- all off the critical path.

The host schedules (batch, query-half) chunk segments into the pair grid
(minimizing total chunks, then pair count, since every core executes the
full slot grid), builds per-core inputs, and sums/normalizes in fp64.
"""

import math
from collections import deque
from contextlib import ExitStack

import ml_dtypes
import numpy as np

import concourse.bacc as bacc
import concourse.mybir as mybir
import concourse.tile as tile
from concourse.bass import AP
from concourse.bass_utils import run_bass_kernel_spmd

N_CORES = 8
B, L, D = 8, 2048, 128
CH = 128          # keys per chunk
WQ = 512          # queries per slot
QT_N = WQ // 128  # PV matmul subtiles per slot (4)
WH = 1024         # queries per pair (two slots)
DV = D + 1        # V columns + ones column
OW = QT_N * DV    # output columns per slot (516)
SCALE = 1.0 / math.sqrt(D)

# Schraudolph fast-exp: bf16 bits of exp(SCALE*st) ~ int16((st + DADD)*CMUL).
# CMUL = SCALE * 128/ln2; DADD = (127*128 + ADJ)/CMUL; ADJ centers the
# 2^frac chord (max overshoot ~+6%).
A16 = 128.0 / math.log(2.0)
CMUL = A16 * SCALE
ADJ = -5.9
DADD = (16256.0 + ADJ) / CMUL
NS2 = 896         # ScalarE exp columns per pair unit (of 1024)
NS1 = 448         # ScalarE exp columns per single unit (of 512)

BF16 = ml_dtypes.bfloat16


# ---------------------------------------------------------------- scheduling

def _try_pack(groups, structure, order, n_cores):
    """Cut groups (id, nchunks) into segments placed into bins of the given
    structure (one bin per (core, pair)).  Returns {(core, pair): (gid,
    chunk_start, nchunks)} or None if the groups don't fit."""
    bins = []  # [capacity, core, pair]
    for s, c in enumerate(structure):
        for core in range(n_cores):
            bins.append([c, core, s])
    placement = {}
    for gid, total in order:
        done = 0
        while done < total:
            rem = total - done
            if not bins:
                return None
            bins.sort(key=lambda b: b[0])
            if rem >= bins[-1][0]:
                cap, core, s = bins.pop()
            else:
                i = next(i for i, b in enumerate(bins) if b[0] >= rem)
                cap, core, s = bins.pop(i)
            take = min(cap, rem)
            placement[(core, s)] = (gid, done, take)
            done += take
    return placement


def _schedule(valid_lens):
    """Choose a pair structure [C_1..C_P] (identical on every core) and an
    assignment of (batch, query-half) chunk segments to (core, pair).
    Pair p expands to slots 2p/2p+1 sharing one K/V block.  Cost model:
    every core executes the full grid, so minimize total chunks first,
    then the number of pairs (each slot boundary costs drain work)."""
    import random

    nk = [max(1, -(-int(v) // CH)) for v in valid_lens]
    groups = []  # gid -> (b, qh, nchunks)
    for b in range(B):
        for qh in range(L // WH):
            groups.append((b, qh, nk[b]))
    sizes = [(gid, g[2]) for gid, g in enumerate(groups)]
    t_all = sum(s for _, s in sizes)
    tpc0 = max(1, -(-t_all // N_CORES))
    rng = random.Random(0)

    def partitions(n, max_parts):
        def rec(n, maxval, parts):
            if n == 0:
                yield list(parts)
                return
            if len(parts) == max_parts:
                return
            for v in range(min(n, maxval), 0, -1):
                parts.append(v)
                yield from rec(n - v, v, parts)
                parts.pop()

        yield from rec(n, n, [])

    # cost = total half-chunks (every core runs them) + boundary drain cost
    best = None  # (cost, structure, placement)
    for tpc in range(tpc0, tpc0 + max(nk) + 3):
        if best is not None and 2 * tpc * 429 > best[0]:
            break
        for maxp in (3, 4, 5):
            for structure in partitions(tpc, maxp):
                orders = [sorted(sizes, key=lambda x: -x[1])]
                for _ in range(300):
                    o = sizes[:]
                    rng.shuffle(o)
                    orders.append(o)
                for order in orders:
                    placement = _try_pack(groups, structure, order, N_CORES)
                    if placement is not None:
                        cost = 2 * tpc * 429 + sum(
                            max(0, 1450 - 170 * c) for c in structure
                            for _ in range(2))
                        if best is None or cost < best[0]:
                            best = (cost, structure, placement)
                        break
    assert best is not None
    _, structure, placement = best
    passign = [[None] * len(structure) for _ in range(N_CORES)]
    for (core, s), (gid, start, n) in placement.items():
        b, qh, _ = groups[gid]
        passign[core][s] = (b, qh, start, n)
    # ascending pair size: small pairs first (their input lands first, the
    # pipeline starts early), big pairs last (their inputs have time to
    # arrive while earlier slots compute).
    order = sorted(range(len(structure)), key=lambda s: structure[s])
    structure = [structure[s] for s in order]
    passign = [[row[s] for s in order] for row in passign]
    return structure, passign


# ------------------------------------------------------------- device program

def _pair_layout(structure):
    """Per-pair combined input layout: [qtE | qtO | kt | vx] in one bf16
    buffer.  Returns (offsets, total_width): offsets[p] = (qt_base, kt_off,
    vx_off)."""
    offsets = []
    base = 0
    for C in structure:
        offsets.append((base, base + WH, base + WH + C * CH))
        base += WH + C * (CH + DV)
    return offsets, base


def _build_program(structure):
    P = len(structure)           # pairs
    S = 2 * P                    # slots
    offsets, totw = _pair_layout(structure)
    slot_g0 = []                 # pt base chunk index per slot
    acc = 0
    for C in structure:
        slot_g0.extend([acc, acc + C])
        acc += 2 * C
    T = acc                      # total half-chunks
    nc = bacc.Bacc("TRN2", target_bir_lowering=False, debug=False)
    data_d = nc.dram_tensor("data", [128, totw], mybir.dt.bfloat16,
                            kind="ExternalInput").ap()
    out_d = nc.dram_tensor("out", [S * 128, OW], mybir.dt.bfloat16,
                           kind="ExternalOutput").ap()

    with tile.TileContext(nc) as tc, ExitStack() as ctx:
        sb_pool = ctx.enter_context(tc.tile_pool(name="sb", bufs=1))
        st_pool = ctx.enter_context(tc.tile_pool(name="st", bufs=2,
                                                 space="PSUM"))
        pv_pool = ctx.enter_context(tc.tile_pool(name="pv", bufs=1,
                                                 space="PSUM"))
        stage_pool = ctx.enter_context(tc.tile_pool(name="stage", bufs=2))

        data_sb = sb_pool.tile([128, totw], mybir.dt.bfloat16)
        pt_sb = sb_pool.tile([128, T * WQ], mybir.dt.bfloat16)

        # gpsimd queue: warmup memset + Schraudolph multiplier const first
        warm_sb = sb_pool.tile([128, 512], mybir.dt.bfloat16)
        nc.gpsimd.memset(warm_sb[:], 0.0)
        cmul_sb = sb_pool.tile([128, 1], mybir.dt.float32)
        nc.gpsimd.memset(cmul_sb[:], CMUL)

        # input DMAs all on the sync queue, in consumption order (a single
        # queue delivers in order; multiple queues round-robin on the wire
        # and starve the early slots).  Big pairs split qt+kt / vx.
        for p, C in enumerate(structure):
            base, kt0, vx0 = offsets[p]
            end = vx0 + C * DV
            if p == 0:
                h0 = kt0 + min(C, 2) * CH
                nc.sync.dma_start(data_sb[:, base:h0], data_d[:, base:h0])
                nc.sync.dma_start(data_sb[:, h0:end], data_d[:, h0:end])
            elif C <= 2:
                nc.sync.dma_start(data_sb[:, base:end], data_d[:, base:end])
            else:
                nc.sync.dma_start(data_sb[:, base:vx0], data_d[:, base:vx0])
                nc.sync.dma_start(data_sb[:, vx0:end], data_d[:, vx0:end])

        # PE warmup: dummy matmuls during the initial DMA wait ramp the PE
        # clock toward 2.4 GHz before real work.
        warm_ps = st_pool.tile([128, WH], mybir.dt.float32, tag="st")
        for _ in range(5):
            nc.tensor.matmul(warm_ps[:, 0:512], warm_sb[:, 0:128],
                             warm_sb[:])

        # pv accumulators: bank A per slot parity (j0-2 packed), bank B
        # shared (j3-even at cols 0:DV, j3-odd at DV:2*DV); groups
        # accumulate start=False onto pre-zeroed ranges (matmul start=True
        # resets a whole PSUM bank, so packed groups cannot use it).
        pv_a0 = pv_pool.tile([128, 512], mybir.dt.float32)
        pv_a1 = pv_pool.tile([128, 512], mybir.dt.float32)
        pv_b = pv_pool.tile([128, 512], mybir.dt.float32)

        def pv_ranges(s):
            a = pv_a0 if s % 2 == 0 else pv_a1
            b0 = (s % 2) * DV
            return a[:, 0:3 * DV], pv_b[:, b0:b0 + DV]

        for s0 in range(min(2, S)):
            pa0, pb0 = pv_ranges(s0)
            nc.vector.memset(pa0, 0.0)
            nc.vector.memset(pb0, 0.0)

        # exp units: (slot, c0, nchunks) with nchunks in {1, 2}
        units = []
        for s in range(S):
            C = structure[s // 2]
            c = 0
            while c < C:
                n = 2 if c + 1 < C else 1
                units.append((s, c, n))
                c += n

        pending = deque()
        out_q = [nc.sync, nc.gpsimd]

        def emit_front(s, c0, n):
            p = s // 2
            base, kt0, vx0 = offsets[p]
            qt = data_sb[:, base + (s % 2) * WQ:base + (s % 2) * WQ + WQ]
            st = st_pool.tile([128, WH], mybir.dt.float32, tag="st")
            for i in range(n):
                kt = data_sb[:, kt0 + (c0 + i) * CH:kt0 + (c0 + i + 1) * CH]
                nc.tensor.matmul(st[:, i * WQ:(i + 1) * WQ], kt, qt)
            w = n * WQ
            ns = NS2 if n == 2 else NS1
            p0 = (slot_g0[s] + c0) * WQ
            nc.scalar.activation(pt_sb[:, p0:p0 + ns], st[:, 0:ns],
                                 mybir.ActivationFunctionType.Exp,
                                 bias=0.0, scale=SCALE)
            if w > ns:
                cm = cmul_sb[:, 0:1]
                cbc = AP(cm.tensor, cm.offset,
                         [[cm.ap[0][0], 128], [0, w - ns]])
                nc.vector.scalar_tensor_tensor(
                    pt_sb[:, p0 + ns:p0 + w].bitcast(mybir.dt.int16),
                    st[:, ns:w], DADD, cbc,
                    mybir.AluOpType.add, mybir.AluOpType.mult)

        def emit_back(s, c0, n):
            p = s // 2
            base, kt0, vx0 = offsets[p]
            pa, pb = pv_ranges(s)
            C = structure[p]
            for i in range(n):
                c = c0 + i
                p0 = (slot_g0[s] + c) * WQ
                vx = data_sb[:, vx0 + c * DV:vx0 + (c + 1) * DV]
                for j in range(QT_N):
                    pv = pa[:, j * DV:(j + 1) * DV] if j < 3 else pb
                    nc.tensor.matmul(
                        pv, pt_sb[:, p0 + j * 128:p0 + (j + 1) * 128],
                        vx, start=False, stop=(c == C - 1),
                        skip_group_check=True)
            if c0 + n == C:
                # both drain copies ride the Vector engine (ScalarE is the
                # exp-cadence setter); the output DMA goes out in two pieces
                # so each transfer starts as soon as its copy lands
                stage = stage_pool.tile([128, OW], mybir.dt.bfloat16)
                nc.vector.tensor_copy(stage[:, 0:3 * DV], pa)
                out_q[s % 2].dma_start(out_d[s * 128:(s + 1) * 128, 0:3 * DV],
                                       stage[:, 0:3 * DV])
                nc.vector.tensor_copy(stage[:, 3 * DV:OW], pb)
                out_q[s % 2].dma_start(out_d[s * 128:(s + 1) * 128, 3 * DV:OW],
                                       stage[:, 3 * DV:OW])
                if s + 2 < S:
                    # re-zero for the slot that reuses these banks; emitted
                    # after the copies so accumulate -> copy -> zero ->
                    # accumulate is the program order
                    na, nb = pv_ranges(s + 2)
                    nc.vector.memset(na, 0.0)
                    nc.vector.memset(nb, 0.0)

        for u in units:
            emit_front(*u)
            pending.append(u)
            if len(pending) > 2:
                emit_back(*pending.popleft())
        while pending:
            emit_back(*pending.popleft())
    nc.compile()
    return nc


# ------------------------------------------------------------------- kernel

def _prep_inputs(queries, keys, values, valid_lens, structure, passign):
    offsets, totw = _pair_layout(structure)
    karange = np.arange(CH)
    in_maps = []
    for core in range(N_CORES):
        data = np.zeros((128, totw), dtype=BF16)
        for p, C in enumerate(structure):
            seg = passign[core][p]
            if seg is None:
                continue
            b, qh, cstart, ncr = seg
            base, kt0, vx0 = offsets[p]
            data[:, base:base + WH] = queries[b, qh * WH:(qh + 1) * WH, :].T
            for ci in range(ncr):
                k0 = (cstart + ci) * CH
                valid = (k0 + karange) < int(valid_lens[b])
                data[:, kt0 + ci * CH:kt0 + (ci + 1) * CH] = \
                    keys[b, k0:k0 + CH, :].T
                data[:, vx0 + ci * DV:vx0 + ci * DV + D] = \
                    values[b, k0:k0 + CH, :] * valid[:, None]
                data[:, vx0 + ci * DV + D] = valid
        in_maps.append({"data": data})
    return in_maps


def _gather(results, structure, passign):
    S = 2 * len(structure)
    num = np.zeros((B, L, D), dtype=np.float64)
    den = np.zeros((B, L), dtype=np.float64)
    for core in range(N_CORES):
        out = np.asarray(results[core]["out"], dtype=np.float64)
        out = out.reshape(S, 128, OW)
        for p in range(len(structure)):
            seg = passign[core][p]
            if seg is None:
                continue
            b, qh, _, _ = seg
            for half in range(2):
                s = 2 * p + half
                for j in range(QT_N):
                    q0 = qh * WH + half * WQ + j * 128
                    rows = slice(q0, q0 + 128)
                    num[b, rows, :] += out[s, :, j * DV:j * DV + D]
                    den[b, rows] += out[s, :, j * DV + D]
    return (num / den[:, :, None]).astype(np.float32)


def kernel(queries, keys, values, valid_lens):
    queries = np.asarray(queries, dtype=np.float32)
    keys = np.asarray(keys, dtype=np.float32)
    values = np.asarray(values, dtype=np.float32)
    valid_lens = np.asarray(valid_lens, dtype=np.int32)

    structure, passign = _schedule(valid_lens)
    nc = _build_program(structure)
    in_maps = _prep_inputs(queries, keys, values, valid_lens, structure,
                           passign)
    res = run_bass_kernel_spmd(nc, in_maps, core_ids=list(range(N_CORES)))
    return _gather(res.results, structure, passign)


# revision 28
# speedup vs baseline: 1.0006x; 1.0006x over previous
"""Masked dot-product attention on 8 Trainium2 NeuronCores.

Full inputs: queries/keys/values [8, 2048, 128] f32, valid_lens [8] i32.
Output: softmax(Q K^T / sqrt(128), masked to valid_lens) @ V, [8, 2048, 128] f32.

Strategy
--------
Keys at positions >= valid_lens[b] carry zero softmax weight, so only
ceil(vl[b]/128) key-chunks per batch matter.  Scores are O(6), so softmax
needs no max-subtraction and partial (numerator, denominator) sums over
disjoint key ranges are additive -- work splits across cores and is
recombined on the host.  Masking is applied entirely on the V side: the
host zeroes V rows and the ones-column for invalid keys, so their
(finite, garbage) exp weights contribute exactly 0 to both numerator and
denominator and the exponentials need no masking at all.

The device program is a flat software pipeline over "half-chunks"
(128 keys x 512 queries).  Half-chunks are grouped into slots of 512
queries; slots come in PAIRS covering the two query-halves of one
(batch, key-range) segment, sharing one K^T/V input block (halves input
DMA).  Adjacent chunks form exp UNITS of two (one odd single per slot)
so the fixed per-instruction PSUM-access cost of the exponent engines is
paid once per 2 chunks.  Per unit:
  S^T   = K_chunk @ Q^T               2 matmuls -> one [128k x 1024q]
                                      2-bank PSUM tile
  P^T   = exp(SCALE*S^T)              ScalarE native exp on 896 cols,
          VectorE Schraudolph fast-exp on 128 cols (one
          scalar_tensor_tensor producing bf16 bit patterns in int16)
  PV   += P^T_j^T @ [V_chunk | 1]     4 matmuls (129 cols) per chunk,
          accumulating with start=False onto pre-zeroed PSUM ranges
          (matmul start=True resets a whole bank, so packed groups
          cannot use it); ones-column = softmax denominator
pv bank A (per slot parity) holds j0-2, bank B is shared (j3 of even /
odd slots at different columns): 3 pv banks + 4 st banks.  At slot end
the two pv ranges are copied (Vector + Scalar) to a bf16 stage tile,
DMA'd partition-major (contiguous 1032B rows), and the banks re-zeroed
for the slot two ahead -- all off the critical path.

The host schedules (batch, query-half) chunk segments into the pair grid
(minimizing total chunks, then pair count, since every core executes the
full slot grid), builds per-core inputs, and sums/normalizes in fp64.
"""

import math
from collections import deque
from contextlib import ExitStack

import ml_dtypes
import numpy as np

import concourse.bacc as bacc
import concourse.mybir as mybir
import concourse.tile as tile
from concourse.bass import AP
from concourse.bass_utils import run_bass_kernel_spmd

N_CORES = 8
B, L, D = 8, 2048, 128
CH = 128          # keys per chunk
WQ = 512          # queries per slot
QT_N = WQ // 128  # PV matmul subtiles per slot (4)
WH = 1024         # queries per pair (two slots)
DV = D + 1        # V columns + ones column
OW = QT_N * DV    # output columns per slot (516)
SCALE = 1.0 / math.sqrt(D)

# Schraudolph fast-exp: bf16 bits of exp(SCALE*st) ~ int16((st + DADD)*CMUL).
# CMUL = SCALE * 128/ln2; DADD = (127*128 + ADJ)/CMUL; ADJ centers the
# 2^frac chord (max overshoot ~+6%).
A16 = 128.0 / math.log(2.0)
CMUL = A16 * SCALE
ADJ = -5.9
DADD = (16256.0 + ADJ) / CMUL
NS2 = 896         # ScalarE exp columns per pair unit (of 1024)
NS1 = 448         # ScalarE exp columns per single unit (of 512)

BF16 = ml_dtypes.bfloat16


# ---------------------------------------------------------------- scheduling

def _try_pack(groups, structure, order, n_cores):
    """Cut groups (id, nchunks) into segments placed into bins of the given
    structure (one bin per (core, pair)).  Returns {(core, pair): (gid,
    chunk_start, nchunks)} or None if the groups don't fit."""
    bins = []  # [capacity, core, pair]
    for s, c in enumerate(structure):
        for core in range(n_cores):
            bins.append([c, core, s])
    placement = {}
    for gid, total in order:
        done = 0
        while done < total:
            rem = total - done
            if not bins:
                return None
            bins.sort(key=lambda b: b[0])
            if rem >= bins[-1][0]:
                cap, core, s = bins.pop()
            else:
                i = next(i for i, b in enumerate(bins) if b[0] >= rem)
                cap, core, s = bins.pop(i)
            take = min(cap, rem)
            placement[(core, s)] = (gid, done, take)
            done += take
    return placement


def _schedule(valid_lens):
    """Choose a pair structure [C_1..C_P] (identical on every core) and an
    assignment of (batch, query-half) chunk segments to (core, pair).
    Pair p expands to slots 2p/2p+1 sharing one K/V block.  Cost model:
    every core executes the full grid, so minimize total chunks first,
    then the number of pairs (each slot boundary costs drain work)."""
    import random

    nk = [max(1, -(-int(v) // CH)) for v in valid_lens]
    groups = []  # gid -> (b, qh, nchunks)
    for b in range(B):
        for qh in range(L // WH):
            groups.append((b, qh, nk[b]))
    sizes = [(gid, g[2]) for gid, g in enumerate(groups)]
    t_all = sum(s for _, s in sizes)
    tpc0 = max(1, -(-t_all // N_CORES))
    rng = random.Random(0)

    def partitions(n, max_parts):
        def rec(n, maxval, parts):
            if n == 0:
                yield list(parts)
                return
            if len(parts) == max_parts:
                return
            for v in range(min(n, maxval), 0, -1):
                parts.append(v)
                yield from rec(n - v, v, parts)
                parts.pop()

        yield from rec(n, n, [])

    # cost = total half-chunks (every core runs them) + boundary drain cost
    best = None  # (cost, structure, placement)
    for tpc in range(tpc0, tpc0 + max(nk) + 3):
        if best is not None and 2 * tpc * 429 > best[0]:
            break
        for maxp in (3, 4, 5):
            for structure in partitions(tpc, maxp):
                orders = [sorted(sizes, key=lambda x: -x[1])]
                for _ in range(300):
                    o = sizes[:]
                    rng.shuffle(o)
                    orders.append(o)
                for order in orders:
                    placement = _try_pack(groups, structure, order, N_CORES)
                    if placement is not None:
                        cost = 2 * tpc * 429 + sum(
                            max(0, 1450 - 170 * c) for c in structure
                            for _ in range(2))
                        if best is None or cost < best[0]:
                            best = (cost, structure, placement)
                        break
    assert best is not None
    _, structure, placement = best
    passign = [[None] * len(structure) for _ in range(N_CORES)]
    for (core, s), (gid, start, n) in placement.items():
        b, qh, _ = groups[gid]
        passign[core][s] = (b, qh, start, n)
    # ascending pair size: small pairs first (their input lands first, the
    # pipeline starts early), big pairs last (their inputs have time to
    # arrive while earlier slots compute).
    order = sorted(range(len(structure)), key=lambda s: structure[s])
    structure = [structure[s] for s in order]
    passign = [[row[s] for s in order] for row in passign]
    return structure, passign


# ------------------------------------------------------------- device program

def _pair_layout(structure):
    """Per-pair combined input layout: [qtE | qtO | kt | vx] in one bf16
    buffer.  Returns (offsets, total_width): offsets[p] = (qt_base, kt_off,
    vx_off)."""
    offsets = []
    base = 0
    for C in structure:
        offsets.append((base, base + WH, base + WH + C * CH))
        base += WH + C * (CH + DV)
    return offsets, base


def _build_program(structure):
    P = len(structure)           # pairs
    S = 2 * P                    # slots
    offsets, totw = _pair_layout(structure)
    slot_g0 = []                 # pt base chunk index per slot
    acc = 0
    for C in structure:
        slot_g0.extend([acc, acc + C])
        acc += 2 * C
    T = acc                      # total half-chunks
    nc = bacc.Bacc("TRN2", target_bir_lowering=False, debug=False)
    data_d = nc.dram_tensor("data", [128, totw], mybir.dt.bfloat16,
                            kind="ExternalInput").ap()
    out_d = nc.dram_tensor("out", [S * 128, OW], mybir.dt.bfloat16,
                           kind="ExternalOutput").ap()

    with tile.TileContext(nc) as tc, ExitStack() as ctx:
        sb_pool = ctx.enter_context(tc.tile_pool(name="sb", bufs=1))
        st_pool = ctx.enter_context(tc.tile_pool(name="st", bufs=2,
                                                 space="PSUM"))
        pv_pool = ctx.enter_context(tc.tile_pool(name="pv", bufs=1,
                                                 space="PSUM"))
        stage_pool = ctx.enter_context(tc.tile_pool(name="stage", bufs=2))

        data_sb = sb_pool.tile([128, totw], mybir.dt.bfloat16)
        pt_sb = sb_pool.tile([128, T * WQ], mybir.dt.bfloat16)

        # gpsimd queue: warmup memset + Schraudolph multiplier const first
        warm_sb = sb_pool.tile([128, 512], mybir.dt.bfloat16)
        nc.gpsimd.memset(warm_sb[:], 0.0)
        cmul_sb = sb_pool.tile([128, 1], mybir.dt.float32)
        nc.gpsimd.memset(cmul_sb[:], CMUL)

        # input DMAs all on the sync queue, in consumption order (a single
        # queue delivers in order; multiple queues round-robin on the wire
        # and starve the early slots).  Big pairs split qt+kt / vx.
        for p, C in enumerate(structure):
            base, kt0, vx0 = offsets[p]
            end = vx0 + C * DV
            if p == 0:
                h0 = kt0 + min(C, 2) * CH
                nc.sync.dma_start(data_sb[:, base:h0], data_d[:, base:h0])
                nc.sync.dma_start(data_sb[:, h0:end], data_d[:, h0:end])
            elif C <= 2:
                nc.sync.dma_start(data_sb[:, base:end], data_d[:, base:end])
            else:
                nc.sync.dma_start(data_sb[:, base:vx0], data_d[:, base:vx0])
                nc.sync.dma_start(data_sb[:, vx0:end], data_d[:, vx0:end])

        # PE warmup: dummy matmuls during the initial DMA wait ramp the PE
        # clock toward 2.4 GHz before real work.
        warm_ps = st_pool.tile([128, WH], mybir.dt.float32, tag="st")
        for _ in range(5):
            nc.tensor.matmul(warm_ps[:, 0:512], warm_sb[:, 0:128],
                             warm_sb[:])

        # pv accumulators: bank A per slot parity (j0-2 packed), bank B
        # shared (j3-even at cols 0:DV, j3-odd at DV:2*DV); groups
        # accumulate start=False onto pre-zeroed ranges (matmul start=True
        # resets a whole PSUM bank, so packed groups cannot use it).
        pv_a0 = pv_pool.tile([128, 512], mybir.dt.float32)
        pv_a1 = pv_pool.tile([128, 512], mybir.dt.float32)
        pv_b = pv_pool.tile([128, 512], mybir.dt.float32)

        def pv_ranges(s):
            a = pv_a0 if s % 2 == 0 else pv_a1
            b0 = (s % 2) * DV
            return a[:, 0:3 * DV], pv_b[:, b0:b0 + DV]

        for s0 in range(min(2, S)):
            pa0, pb0 = pv_ranges(s0)
            nc.vector.memset(pa0, 0.0)
            nc.vector.memset(pb0, 0.0)

        # exp units: (slot, c0, nchunks) with nchunks in {1, 2}
        units = []
        for s in range(S):
            C = structure[s // 2]
            c = 0
            while c < C:
                n = 2 if c + 1 < C else 1
                units.append((s, c, n))
                c += n

        pending = deque()
        out_q = [nc.sync, nc.gpsimd]

        def emit_front(s, c0, n):
            p = s // 2
            base, kt0, vx0 = offsets[p]
            qt = data_sb[:, base + (s % 2) * WQ:base + (s % 2) * WQ + WQ]
            st = st_pool.tile([128, WH], mybir.dt.float32, tag="st")
            for i in range(n):
                kt = data_sb[:, kt0 + (c0 + i) * CH:kt0 + (c0 + i + 1) * CH]
                nc.tensor.matmul(st[:, i * WQ:(i + 1) * WQ], kt, qt)
            w = n * WQ
            ns = NS2 if n == 2 else NS1
            p0 = (slot_g0[s] + c0) * WQ
            nc.scalar.activation(pt_sb[:, p0:p0 + ns], st[:, 0:ns],
                                 mybir.ActivationFunctionType.Exp,
                                 bias=0.0, scale=SCALE)
            if w > ns:
                cm = cmul_sb[:, 0:1]
                cbc = AP(cm.tensor, cm.offset,
                         [[cm.ap[0][0], 128], [0, w - ns]])
                nc.vector.scalar_tensor_tensor(
                    pt_sb[:, p0 + ns:p0 + w].bitcast(mybir.dt.int16),
                    st[:, ns:w], DADD, cbc,
                    mybir.AluOpType.add, mybir.AluOpType.mult)

        def emit_back(s, c0, n):
            p = s // 2
            base, kt0, vx0 = offsets[p]
            pa, pb = pv_ranges(s)
            C = structure[p]
            for i in range(n):
                c = c0 + i
                p0 = (slot_g0[s] + c) * WQ
                vx = data_sb[:, vx0 + c * DV:vx0 + (c + 1) * DV]
                for j in range(QT_N):
                    pv = pa[:, j * DV:(j + 1) * DV] if j < 3 else pb
                    nc.tensor.matmul(
                        pv, pt_sb[:, p0 + j * 128:p0 + (j + 1) * 128],
                        vx, start=False, stop=(c == C - 1),
                        skip_group_check=True)
            if c0 + n == C:
                stage = stage_pool.tile([128, OW], mybir.dt.bfloat16)
                nc.vector.tensor_copy(stage[:, 0:3 * DV], pa)
                nc.scalar.copy(stage[:, 3 * DV:OW], pb)
                out_q[s % 2].dma_start(out_d[s * 128:(s + 1) * 128, :],
                                       stage[:])
                if s + 2 < S:
                    # re-zero for the slot that reuses these banks; emitted
                    # after the copies so accumulate -> copy -> zero ->
                    # accumulate is the program order
                    na, nb = pv_ranges(s + 2)
                    nc.vector.memset(na, 0.0)
                    nc.vector.memset(nb, 0.0)

        for u in units:
            emit_front(*u)
            pending.append(u)
            if len(pending) > 2:
                emit_back(*pending.popleft())
        while pending:
            emit_back(*pending.popleft())
    nc.compile()
    return nc


# ------------------------------------------------------------------- kernel

def _prep_inputs(queries, keys, values, valid_lens, structure, passign):
    offsets, totw = _pair_layout(structure)
    karange = np.arange(CH)
    in_maps = []
    for core in range(N_CORES):
        data = np.zeros((128, totw), dtype=BF16)
        for p, C in enumerate(structure):
            seg = passign[core][p]
            if seg is None:
                continue
            b, qh, cstart, ncr = seg
            base, kt0, vx0 = offsets[p]
            data[:, base:base + WH] = queries[b, qh * WH:(qh + 1) * WH, :].T
            for ci in range(ncr):
                k0 = (cstart + ci) * CH
                valid = (k0 + karange) < int(valid_lens[b])
                data[:, kt0 + ci * CH:kt0 + (ci + 1) * CH] = \
                    keys[b, k0:k0 + CH, :].T
                data[:, vx0 + ci * DV:vx0 + ci * DV + D] = \
                    values[b, k0:k0 + CH, :] * valid[:, None]
                data[:, vx0 + ci * DV + D] = valid
        in_maps.append({"data": data})
    return in_maps


def _gather(results, structure, passign):
    S = 2 * len(structure)
    num = np.zeros((B, L, D), dtype=np.float64)
    den = np.zeros((B, L), dtype=np.float64)
    for core in range(N_CORES):
        out = np.asarray(results[core]["out"], dtype=np.float64)
        out = out.reshape(S, 128, OW)
        for p in range(len(structure)):
            seg = passign[core][p]
            if seg is None:
                continue
            b, qh, _, _ = seg
            for half in range(2):
                s = 2 * p + half
                for j in range(QT_N):
                    q0 = qh * WH + half * WQ + j * 128
                    rows = slice(q0, q0 + 128)
                    num[b, rows, :] += out[s, :, j * DV:j * DV + D]
                    den[b, rows] += out[s, :, j * DV + D]
    return (num / den[:, :, None]).astype(np.float32)


def kernel(queries, keys, values, valid_lens):
    queries = np.asarray(queries, dtype=np.float32)
    keys = np.asarray(keys, dtype=np.float32)
    values = np.asarray(values, dtype=np.float32)
    valid_lens = np.asarray(valid_lens, dtype=np.int32)

    structure, passign = _schedule(valid_lens)
    nc = _build_program(structure)
    in_maps = _prep_inputs(queries, keys, values, valid_lens, structure,
                           passign)
    res = run_bass_kernel_spmd(nc, in_maps, core_ids=list(range(N_CORES)))
    return _gather(res.results, structure, passign)


# revision 29
# speedup vs baseline: 1.1582x; 1.1574x over previous
"""Masked dot-product attention on 8 Trainium2 NeuronCores.

Full inputs: queries/keys/values [8, 2048, 128] f32, valid_lens [8] i32.
Output: softmax(Q K^T / sqrt(128), masked to valid_lens) @ V, [8, 2048, 128] f32.

Strategy
--------
Keys at positions >= valid_lens[b] carry zero softmax weight, so only
ceil(vl[b]/128) key-chunks per batch matter.  Scores are O(6), so softmax
needs no max-subtraction and partial (numerator, denominator) sums over
disjoint key ranges are additive -- work splits across cores and is
recombined on the host.  Masking is applied entirely on the V side: the
host zeroes V rows and the ones-column for invalid keys, so their
(finite, garbage) exp weights contribute exactly 0 to both numerator and
denominator and the exponentials need no masking at all.

The device program is a flat software pipeline over "half-chunks"
(128 keys x 512 queries).  Half-chunks are grouped into slots of 512
queries; slots come in PAIRS covering the two query-halves of one
(batch, key-range) segment, sharing one K^T/V input block (halves input
DMA).  Adjacent chunks form exp UNITS of two (one odd single per slot)
so the fixed per-instruction PSUM-access cost of the exponent engines is
paid once per 2 chunks.  Per unit:
  S^T   = K_chunk @ Q^T               2 matmuls -> one [128k x 1024q]
                                      2-bank PSUM tile
  P^T   = exp(SCALE*S^T)              ScalarE native exp on 896 cols,
          VectorE Schraudolph fast-exp on 128 cols (one
          scalar_tensor_tensor producing bf16 bit patterns in int16)
  PV   += P^T_j^T @ [V_chunk | 1]     4 matmuls (129 cols) per chunk,
          accumulating with start=False onto pre-zeroed PSUM ranges
          (matmul start=True resets a whole bank, so packed groups
          cannot use it); ones-column = softmax denominator
pv bank A (per slot parity) holds j0-2, bank B is shared (j3 of even /
odd slots at different columns): 3 pv banks + 4 st banks.  At slot end
the two pv ranges are copied (Vector + Scalar) to a bf16 stage tile,
DMA'd partition-major (contiguous 1032B rows), and the banks re-zeroed
for the slot two ahead -- all off the critical path.

The host schedules (batch, query-half) chunk segments into the pair grid
(minimizing total chunks, then pair count, since every core executes the
full slot grid), builds per-core inputs, and sums/normalizes in fp64.
"""

import math
from collections import deque
from contextlib import ExitStack

import ml_dtypes
import numpy as np

import concourse.bacc as bacc
import concourse.mybir as mybir
import concourse.tile as tile
from concourse.bass import AP
from concourse.bass_utils import run_bass_kernel_spmd

N_CORES = 8
B, L, D = 8, 2048, 128
CH = 128          # keys per chunk
WQ = 512          # queries per slot
QT_N = WQ // 128  # PV matmul subtiles per slot (4)
WH = 1024         # queries per pair (two slots)
DV = D + 1        # V columns + ones column
OW = QT_N * DV    # output columns per slot (516)
SCALE = 1.0 / math.sqrt(D)

# Schraudolph fast-exp: bf16 bits of exp(SCALE*st) ~ int16((st + DADD)*CMUL).
# CMUL = SCALE * 128/ln2; DADD = (127*128 + ADJ)/CMUL; ADJ centers the
# 2^frac chord (max overshoot ~+6%).
A16 = 128.0 / math.log(2.0)
CMUL = A16 * SCALE
ADJ = -5.9
DADD = (16256.0 + ADJ) / CMUL
NS2 = 896         # ScalarE exp columns per pair unit (of 1024)
NS1 = 448         # ScalarE exp columns per single unit (of 512)

BF16 = ml_dtypes.bfloat16


# ---------------------------------------------------------------- scheduling

def _try_pack(groups, structure, order, n_cores):
    """Cut groups (id, nchunks) into segments placed into bins of the given
    structure (one bin per (core, pair)).  Returns {(core, pair): (gid,
    chunk_start, nchunks)} or None if the groups don't fit."""
    bins = []  # [capacity, core, pair]
    for s, c in enumerate(structure):
        for core in range(n_cores):
            bins.append([c, core, s])
    placement = {}
    for gid, total in order:
        done = 0
        while done < total:
            rem = total - done
            if not bins:
                return None
            bins.sort(key=lambda b: b[0])
            if rem >= bins[-1][0]:
                cap, core, s = bins.pop()
            else:
                i = next(i for i, b in enumerate(bins) if b[0] >= rem)
                cap, core, s = bins.pop(i)
            take = min(cap, rem)
            placement[(core, s)] = (gid, done, take)
            done += take
    return placement


def _schedule(valid_lens):
    """Choose a pair structure [C_1..C_P] (identical on every core) and an
    assignment of (batch, query-half) chunk segments to (core, pair).
    Pair p expands to slots 2p/2p+1 sharing one K/V block.  Cost model:
    every core executes the full grid, so minimize total chunks first,
    then the number of pairs (each slot boundary costs drain work)."""
    import random

    nk = [max(1, -(-int(v) // CH)) for v in valid_lens]
    groups = []  # gid -> (b, qh, nchunks)
    for b in range(B):
        for qh in range(L // WH):
            groups.append((b, qh, nk[b]))
    sizes = [(gid, g[2]) for gid, g in enumerate(groups)]
    t_all = sum(s for _, s in sizes)
    tpc0 = max(1, -(-t_all // N_CORES))
    rng = random.Random(0)

    def partitions(n, max_parts):
        def rec(n, maxval, parts):
            if n == 0:
                yield list(parts)
                return
            if len(parts) == max_parts:
                return
            for v in range(min(n, maxval), 0, -1):
                parts.append(v)
                yield from rec(n - v, v, parts)
                parts.pop()

        yield from rec(n, n, [])

    # cost = total half-chunks (every core runs them) + boundary drain cost
    best = None  # (cost, structure, placement)
    for tpc in range(tpc0, tpc0 + max(nk) + 3):
        if best is not None and 2 * tpc * 429 > best[0]:
            break
        for maxp in (3, 4, 5):
            for structure in partitions(tpc, maxp):
                orders = [sorted(sizes, key=lambda x: -x[1])]
                for _ in range(300):
                    o = sizes[:]
                    rng.shuffle(o)
                    orders.append(o)
                for order in orders:
                    placement = _try_pack(groups, structure, order, N_CORES)
                    if placement is not None:
                        cost = 2 * tpc * 429 + sum(
                            max(0, 1450 - 170 * c) for c in structure
                            for _ in range(2))
                        if best is None or cost < best[0]:
                            best = (cost, structure, placement)
                        break
    assert best is not None
    _, structure, placement = best
    passign = [[None] * len(structure) for _ in range(N_CORES)]
    for (core, s), (gid, start, n) in placement.items():
        b, qh, _ = groups[gid]
        passign[core][s] = (b, qh, start, n)
    # ascending pair size: small pairs first (their input lands first, the
    # pipeline starts early), big pairs last (their inputs have time to
    # arrive while earlier slots compute).
    order = sorted(range(len(structure)), key=lambda s: structure[s])
    structure = [structure[s] for s in order]
    passign = [[row[s] for s in order] for row in passign]
    return structure, passign


# ------------------------------------------------------------- device program

def _pair_layout(structure):
    """Per-pair combined input layout: [qtE | qtO | kt | vx] in one bf16
    buffer.  Returns (offsets, total_width): offsets[p] = (qt_base, kt_off,
    vx_off)."""
    offsets = []
    base = 0
    for C in structure:
        offsets.append((base, base + WH, base + WH + C * CH))
        base += WH + C * (CH + DV)
    return offsets, base


def _build_program(structure):
    P = len(structure)           # pairs
    S = 2 * P                    # slots
    offsets, totw = _pair_layout(structure)
    slot_g0 = []                 # pt base chunk index per slot
    acc = 0
    for C in structure:
        slot_g0.extend([acc, acc + C])
        acc += 2 * C
    T = acc                      # total half-chunks
    nc = bacc.Bacc("TRN2", target_bir_lowering=False, debug=False)
    data_d = nc.dram_tensor("data", [128, totw], mybir.dt.bfloat16,
                            kind="ExternalInput").ap()
    out_d = nc.dram_tensor("out", [S * 128, OW], mybir.dt.bfloat16,
                           kind="ExternalOutput").ap()

    with tile.TileContext(nc) as tc, ExitStack() as ctx:
        sb_pool = ctx.enter_context(tc.tile_pool(name="sb", bufs=1))
        st_pool = ctx.enter_context(tc.tile_pool(name="st", bufs=2,
                                                 space="PSUM"))
        pv_pool = ctx.enter_context(tc.tile_pool(name="pv", bufs=1,
                                                 space="PSUM"))
        stage_pool = ctx.enter_context(tc.tile_pool(name="stage", bufs=2))

        data_sb = sb_pool.tile([128, totw], mybir.dt.bfloat16)
        pt_sb = sb_pool.tile([128, T * WQ], mybir.dt.bfloat16)

        # gpsimd queue: warmup memset + Schraudolph multiplier const first
        warm_sb = sb_pool.tile([128, 512], mybir.dt.bfloat16)
        nc.gpsimd.memset(warm_sb[:], 0.0)
        cmul_sb = sb_pool.tile([128, 1], mybir.dt.float32)
        nc.gpsimd.memset(cmul_sb[:], CMUL)

        # input DMAs all on the sync queue, in consumption order (a single
        # queue delivers in order; multiple queues round-robin on the wire
        # and starve the early slots).  Big pairs split qt+kt / vx.
        for p, C in enumerate(structure):
            base, kt0, vx0 = offsets[p]
            end = vx0 + C * DV
            if p == 0:
                h0 = kt0 + min(C, 2) * CH
                nc.sync.dma_start(data_sb[:, base:h0], data_d[:, base:h0])
                nc.sync.dma_start(data_sb[:, h0:end], data_d[:, h0:end])
            elif C <= 2:
                nc.sync.dma_start(data_sb[:, base:end], data_d[:, base:end])
            else:
                nc.sync.dma_start(data_sb[:, base:vx0], data_d[:, base:vx0])
                nc.sync.dma_start(data_sb[:, vx0:end], data_d[:, vx0:end])

        # PE warmup: dummy matmuls during the initial DMA wait ramp the PE
        # clock toward 2.4 GHz before real work.
        warm_ps = st_pool.tile([128, WH], mybir.dt.float32, tag="st")
        for _ in range(8):
            nc.tensor.matmul(warm_ps[:, 0:512], warm_sb[:, 0:128],
                             warm_sb[:])

        # pv accumulators: bank A per slot parity (j0-2 packed), bank B
        # shared (j3-even at cols 0:DV, j3-odd at DV:2*DV); groups
        # accumulate start=False onto pre-zeroed ranges (matmul start=True
        # resets a whole PSUM bank, so packed groups cannot use it).
        pv_a0 = pv_pool.tile([128, 512], mybir.dt.float32)
        pv_a1 = pv_pool.tile([128, 512], mybir.dt.float32)
        pv_b = pv_pool.tile([128, 512], mybir.dt.float32)

        def pv_ranges(s):
            a = pv_a0 if s % 2 == 0 else pv_a1
            b0 = (s % 2) * DV
            return a[:, 0:3 * DV], pv_b[:, b0:b0 + DV]

        for s0 in range(min(2, S)):
            pa0, pb0 = pv_ranges(s0)
            nc.vector.memset(pa0, 0.0)
            nc.vector.memset(pb0, 0.0)

        # exp units: (slot, c0, nchunks) with nchunks in {1, 2}
        units = []
        for s in range(S):
            C = structure[s // 2]
            c = 0
            while c < C:
                n = 2 if c + 1 < C else 1
                units.append((s, c, n))
                c += n

        pending = deque()
        out_q = [nc.sync, nc.gpsimd]

        def emit_front(s, c0, n):
            p = s // 2
            base, kt0, vx0 = offsets[p]
            qt = data_sb[:, base + (s % 2) * WQ:base + (s % 2) * WQ + WQ]
            st = st_pool.tile([128, WH], mybir.dt.float32, tag="st")
            for i in range(n):
                kt = data_sb[:, kt0 + (c0 + i) * CH:kt0 + (c0 + i + 1) * CH]
                nc.tensor.matmul(st[:, i * WQ:(i + 1) * WQ], kt, qt)
            w = n * WQ
            ns = NS2 if n == 2 else NS1
            p0 = (slot_g0[s] + c0) * WQ
            nc.scalar.activation(pt_sb[:, p0:p0 + ns], st[:, 0:ns],
                                 mybir.ActivationFunctionType.Exp,
                                 bias=0.0, scale=SCALE)
            if w > ns:
                cm = cmul_sb[:, 0:1]
                cbc = AP(cm.tensor, cm.offset,
                         [[cm.ap[0][0], 128], [0, w - ns]])
                nc.vector.scalar_tensor_tensor(
                    pt_sb[:, p0 + ns:p0 + w].bitcast(mybir.dt.int16),
                    st[:, ns:w], DADD, cbc,
                    mybir.AluOpType.add, mybir.AluOpType.mult)

        def emit_back(s, c0, n):
            p = s // 2
            base, kt0, vx0 = offsets[p]
            pa, pb = pv_ranges(s)
            C = structure[p]
            for i in range(n):
                c = c0 + i
                p0 = (slot_g0[s] + c) * WQ
                vx = data_sb[:, vx0 + c * DV:vx0 + (c + 1) * DV]
                for j in range(QT_N):
                    pv = pa[:, j * DV:(j + 1) * DV] if j < 3 else pb
                    nc.tensor.matmul(
                        pv, pt_sb[:, p0 + j * 128:p0 + (j + 1) * 128],
                        vx, start=False, stop=(c == C - 1),
                        skip_group_check=True)
            if c0 + n == C:
                stage = stage_pool.tile([128, OW], mybir.dt.bfloat16)
                nc.vector.tensor_copy(stage[:, 0:3 * DV], pa)
                nc.scalar.copy(stage[:, 3 * DV:OW], pb)
                out_q[s % 2].dma_start(out_d[s * 128:(s + 1) * 128, :],
                                       stage[:])
                if s + 2 < S:
                    # re-zero for the slot that reuses these banks; emitted
                    # after the copies so accumulate -> copy -> zero ->
                    # accumulate is the program order
                    na, nb = pv_ranges(s + 2)
                    nc.vector.memset(na, 0.0)
                    nc.vector.memset(nb, 0.0)

        for u in units:
            emit_front(*u)
            pending.append(u)
            if len(pending) > 2:
                emit_back(*pending.popleft())
        while pending:
            emit_back(*pending.popleft())
    nc.compile()
    return nc


# ------------------------------------------------------------------- kernel

def _prep_inputs(queries, keys, values, valid_lens, structure, passign):
    offsets, totw = _pair_layout(structure)
    karange = np.arange(CH)
    in_maps = []
    for core in range(N_CORES):
        data = np.zeros((128, totw), dtype=BF16)
        for p, C in enumerate(structure):
            seg = passign[core][p]
            if seg is None:
                continue
            b, qh, cstart, ncr = seg
            base, kt0, vx0 = offsets[p]
            data[:, base:base + WH] = queries[b, qh * WH:(qh + 1) * WH, :].T
            for ci in range(ncr):
                k0 = (cstart + ci) * CH
                valid = (k0 + karange) < int(valid_lens[b])
                data[:, kt0 + ci * CH:kt0 + (ci + 1) * CH] = \
                    keys[b, k0:k0 + CH, :].T
                data[:, vx0 + ci * DV:vx0 + ci * DV + D] = \
                    values[b, k0:k0 + CH, :] * valid[:, None]
                data[:, vx0 + ci * DV + D] = valid
        in_maps.append({"data": data})
    return in_maps


def _gather(results, structure, passign):
    S = 2 * len(structure)
    num = np.zeros((B, L, D), dtype=np.float64)
    den = np.zeros((B, L), dtype=np.float64)
    for core in range(N_CORES):
        out = np.asarray(results[core]["out"], dtype=np.float64)
        out = out.reshape(S, 128, OW)
        for p in range(len(structure)):
            seg = passign[core][p]
            if seg is None:
                continue
            b, qh, _, _ = seg
            for half in range(2):
                s = 2 * p + half
                for j in range(QT_N):
                    q0 = qh * WH + half * WQ + j * 128
                    rows = slice(q0, q0 + 128)
                    num[b, rows, :] += out[s, :, j * DV:j * DV + D]
                    den[b, rows] += out[s, :, j * DV + D]
    return (num / den[:, :, None]).astype(np.float32)


def kernel(queries, keys, values, valid_lens):
    queries = np.asarray(queries, dtype=np.float32)
    keys = np.asarray(keys, dtype=np.float32)
    values = np.asarray(values, dtype=np.float32)
    valid_lens = np.asarray(valid_lens, dtype=np.int32)

    structure, passign = _schedule(valid_lens)
    nc = _build_program(structure)
    in_maps = _prep_inputs(queries, keys, values, valid_lens, structure,
                           passign)
    res = run_bass_kernel_spmd(nc, in_maps, core_ids=list(range(N_CORES)))
    return _gather(res.results, structure, passign)


# revision 31
# speedup vs baseline: 1.1818x; 1.0205x over previous
"""Masked dot-product attention on 8 Trainium2 NeuronCores.

Full inputs: queries/keys/values [8, 2048, 128] f32, valid_lens [8] i32.
Output: softmax(Q K^T / sqrt(128), masked to valid_lens) @ V, [8, 2048, 128] f32.

Strategy
--------
Keys at positions >= valid_lens[b] carry zero softmax weight, so only
ceil(vl[b]/128) key-chunks per batch matter.  Scores are O(6), so softmax
needs no max-subtraction and partial (numerator, denominator) sums over
disjoint key ranges are additive -- work splits across cores and is
recombined on the host.  Masking is applied entirely on the V side: the
host zeroes V rows and the ones-column for invalid keys, so their
(finite, garbage) exp weights contribute exactly 0 to both numerator and
denominator and the exponentials need no masking at all.

The device program is a flat software pipeline over "half-chunks"
(128 keys x 512 queries).  Half-chunks are grouped into slots of 512
queries; slots come in PAIRS covering the two query-halves of one
(batch, key-range) segment, sharing one K^T/V input block (halves input
DMA).  Adjacent chunks form exp UNITS of two (one odd single per slot)
so the fixed per-instruction PSUM-access cost of the exponent engines is
paid once per 2 chunks.  Per unit:
  S^T   = K_chunk @ Q^T               2 matmuls -> one [128k x 1024q]
                                      2-bank PSUM tile
  P^T   = exp(SCALE*S^T)              ScalarE native exp on 896 cols,
          VectorE Schraudolph fast-exp on 128 cols (one
          scalar_tensor_tensor producing bf16 bit patterns in int16)
  PV   += P^T_j^T @ [V_chunk | 1]     4 matmuls (129 cols) per chunk,
          accumulating with start=False onto pre-zeroed PSUM ranges
          (matmul start=True resets a whole bank, so packed groups
          cannot use it); ones-column = softmax denominator
pv bank A (per slot parity) holds j0-2, bank B is shared (j3 of even /
odd slots at different columns): 3 pv banks + 4 st banks.  At slot end
the two pv ranges are copied (Vector + Scalar) to a bf16 stage tile,
DMA'd partition-major (contiguous 1032B rows), and the banks re-zeroed
for the slot two ahead -- all off the critical path.

The host schedules (batch, query-half) chunk segments into the pair grid
(minimizing total chunks, then pair count, since every core executes the
full slot grid), builds per-core inputs, and sums/normalizes in fp64.
"""

import math
from collections import deque
from contextlib import ExitStack

import ml_dtypes
import numpy as np

import concourse.bacc as bacc
import concourse.mybir as mybir
import concourse.tile as tile
from concourse.bass import AP
from concourse.bass_utils import run_bass_kernel_spmd

N_CORES = 8
B, L, D = 8, 2048, 128
CH = 128          # keys per chunk
WQ = 512          # queries per slot
QT_N = WQ // 128  # PV matmul subtiles per slot (4)
WH = 1024         # queries per pair (two slots)
DV = D + 1        # V columns + ones column
OW = QT_N * DV    # output columns per slot (516)
SCALE = 1.0 / math.sqrt(D)

# Schraudolph fast-exp: bf16 bits of exp(SCALE*st) ~ int16((st + DADD)*CMUL).
# CMUL = SCALE * 128/ln2; DADD = (127*128 + ADJ)/CMUL; ADJ centers the
# 2^frac chord (max overshoot ~+6%).
A16 = 128.0 / math.log(2.0)
CMUL = A16 * SCALE
ADJ = -5.9
DADD = (16256.0 + ADJ) / CMUL
NS2 = 896         # ScalarE exp columns per pair unit (of 1024)
NS1 = 448         # ScalarE exp columns per single unit (of 512)

BF16 = ml_dtypes.bfloat16


# ---------------------------------------------------------------- scheduling

def _try_pack(groups, structure, order, n_cores):
    """Cut groups (id, nchunks) into segments placed into bins of the given
    structure (one bin per (core, pair)).  Returns {(core, pair): (gid,
    chunk_start, nchunks)} or None if the groups don't fit."""
    bins = []  # [capacity, core, pair]
    for s, c in enumerate(structure):
        for core in range(n_cores):
            bins.append([c, core, s])
    placement = {}
    for gid, total in order:
        done = 0
        while done < total:
            rem = total - done
            if not bins:
                return None
            bins.sort(key=lambda b: b[0])
            if rem >= bins[-1][0]:
                cap, core, s = bins.pop()
            else:
                i = next(i for i, b in enumerate(bins) if b[0] >= rem)
                cap, core, s = bins.pop(i)
            take = min(cap, rem)
            placement[(core, s)] = (gid, done, take)
            done += take
    return placement


def _schedule(valid_lens):
    """Choose a pair structure [C_1..C_P] (identical on every core) and an
    assignment of (batch, query-half) chunk segments to (core, pair).
    Pair p expands to slots 2p/2p+1 sharing one K/V block.  Cost model:
    every core executes the full grid, so minimize total chunks first,
    then the number of pairs (each slot boundary costs drain work)."""
    import random

    nk = [max(1, -(-int(v) // CH)) for v in valid_lens]
    groups = []  # gid -> (b, qh, nchunks)
    for b in range(B):
        for qh in range(L // WH):
            groups.append((b, qh, nk[b]))
    sizes = [(gid, g[2]) for gid, g in enumerate(groups)]
    t_all = sum(s for _, s in sizes)
    tpc0 = max(1, -(-t_all // N_CORES))
    rng = random.Random(0)

    def partitions(n, max_parts):
        def rec(n, maxval, parts):
            if n == 0:
                yield list(parts)
                return
            if len(parts) == max_parts:
                return
            for v in range(min(n, maxval), 0, -1):
                parts.append(v)
                yield from rec(n - v, v, parts)
                parts.pop()

        yield from rec(n, n, [])

    # cost = total half-chunks (every core runs them) + boundary drain cost
    best = None  # (cost, structure, placement)
    for tpc in range(tpc0, tpc0 + max(nk) + 3):
        if best is not None and 2 * tpc * 429 > best[0]:
            break
        for maxp in (3, 4, 5):
            for structure in partitions(tpc, maxp):
                orders = [sorted(sizes, key=lambda x: -x[1])]
                for _ in range(300):
                    o = sizes[:]
                    rng.shuffle(o)
                    orders.append(o)
                for order in orders:
                    placement = _try_pack(groups, structure, order, N_CORES)
                    if placement is not None:
                        cost = 2 * tpc * 429 + sum(
                            max(0, 1450 - 170 * c) for c in structure
                            for _ in range(2))
                        if best is None or cost < best[0]:
                            best = (cost, structure, placement)
                        break
    assert best is not None
    _, structure, placement = best
    passign = [[None] * len(structure) for _ in range(N_CORES)]
    for (core, s), (gid, start, n) in placement.items():
        b, qh, _ = groups[gid]
        passign[core][s] = (b, qh, start, n)
    # ascending pair size: small pairs first (their input lands first, the
    # pipeline starts early), big pairs last (their inputs have time to
    # arrive while earlier slots compute).
    order = sorted(range(len(structure)), key=lambda s: structure[s])
    structure = [structure[s] for s in order]
    passign = [[row[s] for s in order] for row in passign]
    return structure, passign


# ------------------------------------------------------------- device program

def _pair_layout(structure):
    """Per-pair combined input layout: [qtE | qtO | kt | vx] in one bf16
    buffer.  Returns (offsets, total_width): offsets[p] = (qt_base, kt_off,
    vx_off)."""
    offsets = []
    base = 0
    for C in structure:
        offsets.append((base, base + WH, base + WH + C * CH))
        base += WH + C * (CH + DV)
    return offsets, base


def _build_program(structure):
    P = len(structure)           # pairs
    S = 2 * P                    # slots
    offsets, totw = _pair_layout(structure)
    slot_g0 = []                 # pt base chunk index per slot
    acc = 0
    for C in structure:
        slot_g0.extend([acc, acc + C])
        acc += 2 * C
    T = acc                      # total half-chunks
    nc = bacc.Bacc("TRN2", target_bir_lowering=False, debug=False)
    data_d = nc.dram_tensor("data", [128, totw], mybir.dt.bfloat16,
                            kind="ExternalInput").ap()
    out_d = nc.dram_tensor("out", [S * 128, OW], mybir.dt.bfloat16,
                           kind="ExternalOutput").ap()

    with tile.TileContext(nc) as tc, ExitStack() as ctx:
        sb_pool = ctx.enter_context(tc.tile_pool(name="sb", bufs=1))
        st_pool = ctx.enter_context(tc.tile_pool(name="st", bufs=2,
                                                 space="PSUM"))
        pv_pool = ctx.enter_context(tc.tile_pool(name="pv", bufs=1,
                                                 space="PSUM"))
        stage_pool = ctx.enter_context(tc.tile_pool(name="stage", bufs=2))

        data_sb = sb_pool.tile([128, totw], mybir.dt.bfloat16)
        pt_sb = sb_pool.tile([128, T * WQ], mybir.dt.bfloat16)

        # gpsimd queue: warmup memset + Schraudolph multiplier const first
        # (anything queued before the memset would delay the PE warmup chain)
        warm_sb = sb_pool.tile([128, 512], mybir.dt.bfloat16)
        nc.gpsimd.memset(warm_sb[:], 0.0)
        cmul_sb = sb_pool.tile([128, 1], mybir.dt.float32)
        nc.gpsimd.memset(cmul_sb[:], CMUL)

        # input DMAs all on the sync queue, in consumption order (a single
        # queue delivers in order; multiple queues round-robin on the wire
        # and starve the early slots).  Big pairs split qt+kt / vx.
        for p, C in enumerate(structure):
            base, kt0, vx0 = offsets[p]
            end = vx0 + C * DV
            if p == 0:
                h0 = kt0 + min(C, 2) * CH
                nc.sync.dma_start(data_sb[:, base:h0], data_d[:, base:h0])
                nc.sync.dma_start(data_sb[:, h0:end], data_d[:, h0:end])
            elif C <= 2:
                nc.sync.dma_start(data_sb[:, base:end], data_d[:, base:end])
            else:
                nc.sync.dma_start(data_sb[:, base:vx0], data_d[:, base:vx0])
                nc.sync.dma_start(data_sb[:, vx0:end], data_d[:, vx0:end])

        # PE warmup: dummy matmuls during the initial DMA wait ramp the PE
        # clock toward 2.4 GHz before real work.
        warm_ps = st_pool.tile([128, WH], mybir.dt.float32, tag="st")
        for _ in range(7):
            nc.tensor.matmul(warm_ps[:, 0:512], warm_sb[:, 0:128],
                             warm_sb[:])

        # pv accumulators: bank A per slot parity (j0-2 packed), bank B
        # shared (j3-even at cols 0:DV, j3-odd at DV:2*DV); groups
        # accumulate start=False onto pre-zeroed ranges (matmul start=True
        # resets a whole PSUM bank, so packed groups cannot use it).
        pv_a0 = pv_pool.tile([128, 512], mybir.dt.float32)
        pv_a1 = pv_pool.tile([128, 512], mybir.dt.float32)
        pv_b = pv_pool.tile([128, 512], mybir.dt.float32)

        def pv_ranges(s):
            a = pv_a0 if s % 2 == 0 else pv_a1
            b0 = (s % 2) * DV
            return a[:, 0:3 * DV], pv_b[:, b0:b0 + DV]

        for s0 in range(min(2, S)):
            pa0, pb0 = pv_ranges(s0)
            nc.vector.memset(pa0, 0.0)
            nc.vector.memset(pb0, 0.0)

        # exp units: (slot, c0, nchunks) with nchunks in {1, 2}
        units = []
        for s in range(S):
            C = structure[s // 2]
            c = 0
            while c < C:
                n = 2 if c + 1 < C else 1
                units.append((s, c, n))
                c += n

        pending = deque()
        out_q = [nc.sync, nc.gpsimd]

        def emit_front(s, c0, n):
            p = s // 2
            base, kt0, vx0 = offsets[p]
            qt = data_sb[:, base + (s % 2) * WQ:base + (s % 2) * WQ + WQ]
            st = st_pool.tile([128, WH], mybir.dt.float32, tag="st")
            for i in range(n):
                kt = data_sb[:, kt0 + (c0 + i) * CH:kt0 + (c0 + i + 1) * CH]
                nc.tensor.matmul(st[:, i * WQ:(i + 1) * WQ], kt, qt)
            w = n * WQ
            ns = NS2 if n == 2 else NS1
            p0 = (slot_g0[s] + c0) * WQ
            nc.scalar.activation(pt_sb[:, p0:p0 + ns], st[:, 0:ns],
                                 mybir.ActivationFunctionType.Exp,
                                 bias=0.0, scale=SCALE)
            if w > ns:
                cm = cmul_sb[:, 0:1]
                cbc = AP(cm.tensor, cm.offset,
                         [[cm.ap[0][0], 128], [0, w - ns]])
                nc.vector.scalar_tensor_tensor(
                    pt_sb[:, p0 + ns:p0 + w].bitcast(mybir.dt.int16),
                    st[:, ns:w], DADD, cbc,
                    mybir.AluOpType.add, mybir.AluOpType.mult)

        def emit_back(s, c0, n):
            p = s // 2
            base, kt0, vx0 = offsets[p]
            pa, pb = pv_ranges(s)
            C = structure[p]
            for i in range(n):
                c = c0 + i
                p0 = (slot_g0[s] + c) * WQ
                vx = data_sb[:, vx0 + c * DV:vx0 + (c + 1) * DV]
                for j in range(QT_N):
                    pv = pa[:, j * DV:(j + 1) * DV] if j < 3 else pb
                    nc.tensor.matmul(
                        pv, pt_sb[:, p0 + j * 128:p0 + (j + 1) * 128],
                        vx, start=False, stop=(c == C - 1),
                        skip_group_check=True)
            if c0 + n == C:
                stage = stage_pool.tile([128, OW], mybir.dt.bfloat16)
                nc.vector.tensor_copy(stage[:, 0:3 * DV], pa)
                nc.scalar.copy(stage[:, 3 * DV:OW], pb)
                out_q[s % 2].dma_start(out_d[s * 128:(s + 1) * 128, :],
                                       stage[:])
                if s + 2 < S:
                    # re-zero for the slot that reuses these banks; emitted
                    # after the copies so accumulate -> copy -> zero ->
                    # accumulate is the program order
                    na, nb = pv_ranges(s + 2)
                    nc.vector.memset(na, 0.0)
                    nc.vector.memset(nb, 0.0)

        for u in units:
            emit_front(*u)
            pending.append(u)
            if len(pending) > 2:
                emit_back(*pending.popleft())
        while pending:
            emit_back(*pending.popleft())
    nc.compile()
    return nc


# ------------------------------------------------------------------- kernel

def _prep_inputs(queries, keys, values, valid_lens, structure, passign):
    offsets, totw = _pair_layout(structure)
    karange = np.arange(CH)
    in_maps = []
    for core in range(N_CORES):
        data = np.zeros((128, totw), dtype=BF16)
        for p, C in enumerate(structure):
            seg = passign[core][p]
            if seg is None:
                continue
            b, qh, cstart, ncr = seg
            base, kt0, vx0 = offsets[p]
            data[:, base:base + WH] = queries[b, qh * WH:(qh + 1) * WH, :].T
            for ci in range(ncr):
                k0 = (cstart + ci) * CH
                valid = (k0 + karange) < int(valid_lens[b])
                data[:, kt0 + ci * CH:kt0 + (ci + 1) * CH] = \
                    keys[b, k0:k0 + CH, :].T
                data[:, vx0 + ci * DV:vx0 + ci * DV + D] = \
                    values[b, k0:k0 + CH, :] * valid[:, None]
                data[:, vx0 + ci * DV + D] = valid
        in_maps.append({"data": data})
    return in_maps


def _gather(results, structure, passign):
    S = 2 * len(structure)
    num = np.zeros((B, L, D), dtype=np.float64)
    den = np.zeros((B, L), dtype=np.float64)
    for core in range(N_CORES):
        out = np.asarray(results[core]["out"], dtype=np.float64)
        out = out.reshape(S, 128, OW)
        for p in range(len(structure)):
            seg = passign[core][p]
            if seg is None:
                continue
            b, qh, _, _ = seg
            for half in range(2):
                s = 2 * p + half
                for j in range(QT_N):
                    q0 = qh * WH + half * WQ + j * 128
                    rows = slice(q0, q0 + 128)
                    num[b, rows, :] += out[s, :, j * DV:j * DV + D]
                    den[b, rows] += out[s, :, j * DV + D]
    return (num / den[:, :, None]).astype(np.float32)


def kernel(queries, keys, values, valid_lens):
    queries = np.asarray(queries, dtype=np.float32)
    keys = np.asarray(keys, dtype=np.float32)
    values = np.asarray(values, dtype=np.float32)
    valid_lens = np.asarray(valid_lens, dtype=np.int32)

    structure, passign = _schedule(valid_lens)
    nc = _build_program(structure)
    in_maps = _prep_inputs(queries, keys, values, valid_lens, structure,
                           passign)
    res = run_bass_kernel_spmd(nc, in_maps, core_ids=list(range(N_CORES)))
    return _gather(res.results, structure, passign)


# revision 32
# speedup vs baseline: 1.1887x; 1.0058x over previous
"""Masked dot-product attention on 8 Trainium2 NeuronCores.

Full inputs: queries/keys/values [8, 2048, 128] f32, valid_lens [8] i32.
Output: softmax(Q K^T / sqrt(128), masked to valid_lens) @ V, [8, 2048, 128] f32.

Strategy
--------
Keys at positions >= valid_lens[b] carry zero softmax weight, so only
ceil(vl[b]/128) key-chunks per batch matter.  Scores are O(6), so softmax
needs no max-subtraction and partial (numerator, denominator) sums over
disjoint key ranges are additive -- work splits across cores and is
recombined on the host.  Masking is applied entirely on the V side: the
host zeroes V rows and the ones-column for invalid keys, so their
(finite, garbage) exp weights contribute exactly 0 to both numerator and
denominator and the exponentials need no masking at all.

The device program is a flat software pipeline over "half-chunks"
(128 keys x 512 queries).  Half-chunks are grouped into slots of 512
queries; slots come in PAIRS covering the two query-halves of one
(batch, key-range) segment, sharing one K^T/V input block (halves input
DMA).  Adjacent chunks form exp UNITS of two (one odd single per slot)
so the fixed per-instruction PSUM-access cost of the exponent engines is
paid once per 2 chunks.  Per unit:
  S^T   = K_chunk @ Q^T               2 matmuls -> one [128k x 1024q]
                                      2-bank PSUM tile
  P^T   = exp(SCALE*S^T)              ScalarE native exp on 896 cols,
          VectorE Schraudolph fast-exp on 128 cols (one
          scalar_tensor_tensor producing bf16 bit patterns in int16)
  PV   += P^T_j^T @ [V_chunk | 1]     4 matmuls (129 cols) per chunk,
          accumulating with start=False onto pre-zeroed PSUM ranges
          (matmul start=True resets a whole bank, so packed groups
          cannot use it); ones-column = softmax denominator
pv bank A (per slot parity) holds j0-2, bank B is shared (j3 of even /
odd slots at different columns): 3 pv banks + 4 st banks.  At slot end
the two pv ranges are copied (Vector + Scalar) to a bf16 stage tile,
DMA'd partition-major (contiguous 1032B rows), and the banks re-zeroed
for the slot two ahead -- all off the critical path.

The host schedules (batch, query-half) chunk segments into the pair grid
(minimizing total chunks, then pair count, since every core executes the
full slot grid), builds per-core inputs, and sums/normalizes in fp64.
"""

import math
from collections import deque
from contextlib import ExitStack

import ml_dtypes
import numpy as np

import concourse.bacc as bacc
import concourse.mybir as mybir
import concourse.tile as tile
from concourse.bass import AP
from concourse.bass_utils import run_bass_kernel_spmd

N_CORES = 8
B, L, D = 8, 2048, 128
CH = 128          # keys per chunk
WQ = 512          # queries per slot
QT_N = WQ // 128  # PV matmul subtiles per slot (4)
WH = 1024         # queries per pair (two slots)
DV = D + 1        # V columns + ones column
OW = QT_N * DV    # output columns per slot (516)
SCALE = 1.0 / math.sqrt(D)

# Schraudolph fast-exp: bf16 bits of exp(SCALE*st) ~ int16((st + DADD)*CMUL).
# CMUL = SCALE * 128/ln2; DADD = (127*128 + ADJ)/CMUL; ADJ centers the
# 2^frac chord (max overshoot ~+6%).
A16 = 128.0 / math.log(2.0)
CMUL = A16 * SCALE
ADJ = -5.9
DADD = (16256.0 + ADJ) / CMUL
NS2 = 896         # ScalarE exp columns per pair unit (of 1024)
NS1 = 448         # ScalarE exp columns per single unit (of 512)

BF16 = ml_dtypes.bfloat16


# ---------------------------------------------------------------- scheduling

def _try_pack(groups, structure, order, n_cores):
    """Cut groups (id, nchunks) into segments placed into bins of the given
    structure (one bin per (core, pair)).  Returns {(core, pair): (gid,
    chunk_start, nchunks)} or None if the groups don't fit."""
    bins = []  # [capacity, core, pair]
    for s, c in enumerate(structure):
        for core in range(n_cores):
            bins.append([c, core, s])
    placement = {}
    for gid, total in order:
        done = 0
        while done < total:
            rem = total - done
            if not bins:
                return None
            bins.sort(key=lambda b: b[0])
            if rem >= bins[-1][0]:
                cap, core, s = bins.pop()
            else:
                i = next(i for i, b in enumerate(bins) if b[0] >= rem)
                cap, core, s = bins.pop(i)
            take = min(cap, rem)
            placement[(core, s)] = (gid, done, take)
            done += take
    return placement


def _schedule(valid_lens):
    """Choose a pair structure [C_1..C_P] (identical on every core) and an
    assignment of (batch, query-half) chunk segments to (core, pair).
    Pair p expands to slots 2p/2p+1 sharing one K/V block.  Cost model:
    every core executes the full grid, so minimize total chunks first,
    then the number of pairs (each slot boundary costs drain work)."""
    import random

    nk = [max(1, -(-int(v) // CH)) for v in valid_lens]
    groups = []  # gid -> (b, qh, nchunks)
    for b in range(B):
        for qh in range(L // WH):
            groups.append((b, qh, nk[b]))
    sizes = [(gid, g[2]) for gid, g in enumerate(groups)]
    t_all = sum(s for _, s in sizes)
    tpc0 = max(1, -(-t_all // N_CORES))
    rng = random.Random(0)

    def partitions(n, max_parts):
        def rec(n, maxval, parts):
            if n == 0:
                yield list(parts)
                return
            if len(parts) == max_parts:
                return
            for v in range(min(n, maxval), 0, -1):
                parts.append(v)
                yield from rec(n - v, v, parts)
                parts.pop()

        yield from rec(n, n, [])

    # cost = total half-chunks (every core runs them) + boundary drain cost
    best = None  # (cost, structure, placement)
    for tpc in range(tpc0, tpc0 + max(nk) + 3):
        if best is not None and 2 * tpc * 429 > best[0]:
            break
        for maxp in (3, 4, 5):
            for structure in partitions(tpc, maxp):
                orders = [sorted(sizes, key=lambda x: -x[1])]
                for _ in range(300):
                    o = sizes[:]
                    rng.shuffle(o)
                    orders.append(o)
                for order in orders:
                    placement = _try_pack(groups, structure, order, N_CORES)
                    if placement is not None:
                        cost = 2 * tpc * 429 + sum(
                            max(0, 1450 - 170 * c) for c in structure
                            for _ in range(2))
                        if best is None or cost < best[0]:
                            best = (cost, structure, placement)
                        break
    assert best is not None
    _, structure, placement = best
    passign = [[None] * len(structure) for _ in range(N_CORES)]
    for (core, s), (gid, start, n) in placement.items():
        b, qh, _ = groups[gid]
        passign[core][s] = (b, qh, start, n)
    # ascending pair size: small pairs first (their input lands first, the
    # pipeline starts early), big pairs last (their inputs have time to
    # arrive while earlier slots compute).
    order = sorted(range(len(structure)), key=lambda s: structure[s])
    structure = [structure[s] for s in order]
    passign = [[row[s] for s in order] for row in passign]
    return structure, passign


# ------------------------------------------------------------- device program

def _pair_layout(structure):
    """Per-pair combined input layout: [qtE | qtO | kt | vx] in one bf16
    buffer.  Returns (offsets, total_width): offsets[p] = (qt_base, kt_off,
    vx_off)."""
    offsets = []
    base = 0
    for C in structure:
        offsets.append((base, base + WH, base + WH + C * CH))
        base += WH + C * (CH + DV)
    return offsets, base


def _build_program(structure):
    P = len(structure)           # pairs
    S = 2 * P                    # slots
    offsets, totw = _pair_layout(structure)
    slot_g0 = []                 # pt base chunk index per slot
    acc = 0
    for C in structure:
        slot_g0.extend([acc, acc + C])
        acc += 2 * C
    T = acc                      # total half-chunks
    nc = bacc.Bacc("TRN2", target_bir_lowering=False, debug=False)
    data_d = nc.dram_tensor("data", [128, totw], mybir.dt.bfloat16,
                            kind="ExternalInput").ap()
    out_d = nc.dram_tensor("out", [S * 128, OW], mybir.dt.bfloat16,
                           kind="ExternalOutput").ap()

    with tile.TileContext(nc) as tc, ExitStack() as ctx:
        sb_pool = ctx.enter_context(tc.tile_pool(name="sb", bufs=1))
        st_pool = ctx.enter_context(tc.tile_pool(name="st", bufs=2,
                                                 space="PSUM"))
        pv_pool = ctx.enter_context(tc.tile_pool(name="pv", bufs=1,
                                                 space="PSUM"))
        stage_pool = ctx.enter_context(tc.tile_pool(name="stage", bufs=2))

        data_sb = sb_pool.tile([128, totw], mybir.dt.bfloat16)
        pt_sb = sb_pool.tile([128, T * WQ], mybir.dt.bfloat16)

        # gpsimd queue: warmup memset + Schraudolph multiplier const first
        # (anything queued before the memset would delay the PE warmup chain)
        warm_sb = sb_pool.tile([128, 512], mybir.dt.bfloat16)
        nc.gpsimd.memset(warm_sb[:], 0.0)
        cmul_sb = sb_pool.tile([128, 1], mybir.dt.float32)
        nc.gpsimd.memset(cmul_sb[:], CMUL)

        # input DMAs all on the sync queue, in consumption order (a single
        # queue delivers in order; multiple queues round-robin on the wire
        # and starve the early slots).  Big pairs split qt+kt / vx.
        for p, C in enumerate(structure):
            base, kt0, vx0 = offsets[p]
            end = vx0 + C * DV
            if p == 0:
                h0 = kt0 + min(C, 2) * CH
                nc.sync.dma_start(data_sb[:, base:h0], data_d[:, base:h0])
                nc.sync.dma_start(data_sb[:, h0:end], data_d[:, h0:end])
            elif C <= 2:
                nc.sync.dma_start(data_sb[:, base:end], data_d[:, base:end])
            else:
                nc.sync.dma_start(data_sb[:, base:vx0], data_d[:, base:vx0])
                nc.sync.dma_start(data_sb[:, vx0:end], data_d[:, vx0:end])

        # PE warmup: dummy matmuls during the initial DMA wait ramp the PE
        # clock toward 2.4 GHz before real work.
        warm_ps = st_pool.tile([128, WH], mybir.dt.float32, tag="st")
        for _ in range(7):
            nc.tensor.matmul(warm_ps[:, 0:512], warm_sb[:, 0:128],
                             warm_sb[:])

        # pv accumulators: bank A per slot parity (j0-2 packed), bank B
        # shared (j3-even at cols 0:DV, j3-odd at DV:2*DV); groups
        # accumulate start=False onto pre-zeroed ranges (matmul start=True
        # resets a whole PSUM bank, so packed groups cannot use it).
        pv_a0 = pv_pool.tile([128, 512], mybir.dt.float32)
        pv_a1 = pv_pool.tile([128, 512], mybir.dt.float32)
        pv_b = pv_pool.tile([128, 512], mybir.dt.float32)

        def pv_ranges(s):
            a = pv_a0 if s % 2 == 0 else pv_a1
            b0 = (s % 2) * DV
            return a[:, 0:3 * DV], pv_b[:, b0:b0 + DV]

        for s0 in range(min(2, S)):
            pa0, pb0 = pv_ranges(s0)
            nc.vector.memset(pa0, 0.0)
            nc.vector.memset(pb0, 0.0)

        # exp units: (slot, c0, nchunks) with nchunks in {1, 2}
        units = []
        for s in range(S):
            C = structure[s // 2]
            c = 0
            while c < C:
                n = 2 if c + 1 < C else 1
                units.append((s, c, n))
                c += n

        pending = deque()
        out_q = [nc.sync, nc.gpsimd]

        def emit_front(s, c0, n):
            p = s // 2
            base, kt0, vx0 = offsets[p]
            qt = data_sb[:, base + (s % 2) * WQ:base + (s % 2) * WQ + WQ]
            st = st_pool.tile([128, WH], mybir.dt.float32, tag="st")
            for i in range(n):
                kt = data_sb[:, kt0 + (c0 + i) * CH:kt0 + (c0 + i + 1) * CH]
                nc.tensor.matmul(st[:, i * WQ:(i + 1) * WQ], kt, qt)
            w = n * WQ
            ns = NS2 if n == 2 else NS1
            p0 = (slot_g0[s] + c0) * WQ
            nc.scalar.activation(pt_sb[:, p0:p0 + ns], st[:, 0:ns],
                                 mybir.ActivationFunctionType.Exp,
                                 bias=0.0, scale=SCALE)
            if w > ns:
                cm = cmul_sb[:, 0:1]
                cbc = AP(cm.tensor, cm.offset,
                         [[cm.ap[0][0], 128], [0, w - ns]])
                nc.vector.scalar_tensor_tensor(
                    pt_sb[:, p0 + ns:p0 + w].bitcast(mybir.dt.int16),
                    st[:, ns:w], DADD, cbc,
                    mybir.AluOpType.add, mybir.AluOpType.mult)

        def emit_back(s, c0, n):
            p = s // 2
            base, kt0, vx0 = offsets[p]
            pa, pb = pv_ranges(s)
            C = structure[p]
            for i in range(n):
                c = c0 + i
                p0 = (slot_g0[s] + c) * WQ
                vx = data_sb[:, vx0 + c * DV:vx0 + (c + 1) * DV]
                for j in range(QT_N):
                    pv = pa[:, j * DV:(j + 1) * DV] if j < 3 else pb
                    nc.tensor.matmul(
                        pv, pt_sb[:, p0 + j * 128:p0 + (j + 1) * 128],
                        vx, start=False, stop=(c == C - 1),
                        skip_group_check=True)
            if c0 + n == C:
                stage = stage_pool.tile([128, OW], mybir.dt.bfloat16)
                nc.vector.tensor_copy(stage[:, 0:3 * DV], pa)
                nc.vector.tensor_copy(stage[:, 3 * DV:OW], pb)
                out_q[s % 2].dma_start(out_d[s * 128:(s + 1) * 128, :],
                                       stage[:])
                if s + 2 < S:
                    # re-zero for the slot that reuses these banks; emitted
                    # after the copies so accumulate -> copy -> zero ->
                    # accumulate is the program order
                    na, nb = pv_ranges(s + 2)
                    nc.vector.memset(na, 0.0)
                    nc.vector.memset(nb, 0.0)

        for u in units:
            emit_front(*u)
            pending.append(u)
            if len(pending) > 2:
                emit_back(*pending.popleft())
        while pending:
            emit_back(*pending.popleft())
    nc.compile()
    return nc


# ------------------------------------------------------------------- kernel

def _prep_inputs(queries, keys, values, valid_lens, structure, passign):
    offsets, totw = _pair_layout(structure)
    karange = np.arange(CH)
    in_maps = []
    for core in range(N_CORES):
        data = np.zeros((128, totw), dtype=BF16)
        for p, C in enumerate(structure):
            seg = passign[core][p]
            if seg is None:
                continue
            b, qh, cstart, ncr = seg
            base, kt0, vx0 = offsets[p]
            data[:, base:base + WH] = queries[b, qh * WH:(qh + 1) * WH, :].T
            for ci in range(ncr):
                k0 = (cstart + ci) * CH
                valid = (k0 + karange) < int(valid_lens[b])
                data[:, kt0 + ci * CH:kt0 + (ci + 1) * CH] = \
                    keys[b, k0:k0 + CH, :].T
                data[:, vx0 + ci * DV:vx0 + ci * DV + D] = \
                    values[b, k0:k0 + CH, :] * valid[:, None]
                data[:, vx0 + ci * DV + D] = valid
        in_maps.append({"data": data})
    return in_maps


def _gather(results, structure, passign):
    S = 2 * len(structure)
    num = np.zeros((B, L, D), dtype=np.float64)
    den = np.zeros((B, L), dtype=np.float64)
    for core in range(N_CORES):
        out = np.asarray(results[core]["out"], dtype=np.float64)
        out = out.reshape(S, 128, OW)
        for p in range(len(structure)):
            seg = passign[core][p]
            if seg is None:
                continue
            b, qh, _, _ = seg
            for half in range(2):
                s = 2 * p + half
                for j in range(QT_N):
                    q0 = qh * WH + half * WQ + j * 128
                    rows = slice(q0, q0 + 128)
                    num[b, rows, :] += out[s, :, j * DV:j * DV + D]
                    den[b, rows] += out[s, :, j * DV + D]
    return (num / den[:, :, None]).astype(np.float32)


def kernel(queries, keys, values, valid_lens):
    queries = np.asarray(queries, dtype=np.float32)
    keys = np.asarray(keys, dtype=np.float32)
    values = np.asarray(values, dtype=np.float32)
    valid_lens = np.asarray(valid_lens, dtype=np.int32)

    structure, passign = _schedule(valid_lens)
    nc = _build_program(structure)
    in_maps = _prep_inputs(queries, keys, values, valid_lens, structure,
                           passign)
    res = run_bass_kernel_spmd(nc, in_maps, core_ids=list(range(N_CORES)))
    return _gather(res.results, structure, passign)


# revision 33
# speedup vs baseline: 1.1969x; 1.0069x over previous
"""Masked dot-product attention on 8 Trainium2 NeuronCores.

Full inputs: queries/keys/values [8, 2048, 128] f32, valid_lens [8] i32.
Output: softmax(Q K^T / sqrt(128), masked to valid_lens) @ V, [8, 2048, 128] f32.

Strategy
--------
Keys at positions >= valid_lens[b] carry zero softmax weight, so only
ceil(vl[b]/128) key-chunks per batch matter.  Scores are O(6), so softmax
needs no max-subtraction and partial (numerator, denominator) sums over
disjoint key ranges are additive -- work splits across cores and is
recombined on the host.  Masking is applied entirely on the V side: the
host zeroes V rows and the ones-column for invalid keys, so their
(finite, garbage) exp weights contribute exactly 0 to both numerator and
denominator and the exponentials need no masking at all.

The device program is a flat software pipeline over "half-chunks"
(128 keys x 512 queries).  Half-chunks are grouped into slots of 512
queries; slots come in PAIRS covering the two query-halves of one
(batch, key-range) segment, sharing one K^T/V input block (halves input
DMA).  Adjacent chunks form exp UNITS of two (one odd single per slot)
so the fixed per-instruction PSUM-access cost of the exponent engines is
paid once per 2 chunks.  Per unit:
  S^T   = K_chunk @ Q^T               2 matmuls -> one [128k x 1024q]
                                      2-bank PSUM tile
  P^T   = exp(SCALE*S^T)              ScalarE native exp on 896 cols,
          VectorE Schraudolph fast-exp on 128 cols (one
          scalar_tensor_tensor producing bf16 bit patterns in int16)
  PV   += P^T_j^T @ [V_chunk | 1]     4 matmuls (129 cols) per chunk,
          accumulating with start=False onto pre-zeroed PSUM ranges
          (matmul start=True resets a whole bank, so packed groups
          cannot use it); ones-column = softmax denominator
pv bank A (per slot parity) holds j0-2, bank B is shared (j3 of even /
odd slots at different columns): 3 pv banks + 4 st banks.  At slot end
the two pv ranges are copied (Vector + Scalar) to a bf16 stage tile,
DMA'd partition-major (contiguous 1032B rows), and the banks re-zeroed
for the slot two ahead -- all off the critical path.

The host schedules (batch, query-half) chunk segments into the pair grid
(minimizing total chunks, then pair count, since every core executes the
full slot grid), builds per-core inputs, and sums/normalizes in fp64.
"""

import math
from collections import deque
from contextlib import ExitStack

import ml_dtypes
import numpy as np

import concourse.bacc as bacc
import concourse.mybir as mybir
import concourse.tile as tile
from concourse.bass import AP
from concourse.bass_utils import run_bass_kernel_spmd

N_CORES = 8
B, L, D = 8, 2048, 128
CH = 128          # keys per chunk
WQ = 512          # queries per slot
QT_N = WQ // 128  # PV matmul subtiles per slot (4)
WH = 1024         # queries per pair (two slots)
DV = D + 1        # V columns + ones column
OW = QT_N * DV    # output columns per slot (516)
SCALE = 1.0 / math.sqrt(D)

# Schraudolph fast-exp: bf16 bits of exp(SCALE*st) ~ int16((st + DADD)*CMUL).
# CMUL = SCALE * 128/ln2; DADD = (127*128 + ADJ)/CMUL; ADJ centers the
# 2^frac chord (max overshoot ~+6%).
A16 = 128.0 / math.log(2.0)
CMUL = A16 * SCALE
ADJ = -5.9
DADD = (16256.0 + ADJ) / CMUL
NS2 = 832         # ScalarE exp columns per pair unit (of 1024)
NS1 = 416         # ScalarE exp columns per single unit (of 512)

BF16 = ml_dtypes.bfloat16


# ---------------------------------------------------------------- scheduling

def _try_pack(groups, structure, order, n_cores):
    """Cut groups (id, nchunks) into segments placed into bins of the given
    structure (one bin per (core, pair)).  Returns {(core, pair): (gid,
    chunk_start, nchunks)} or None if the groups don't fit."""
    bins = []  # [capacity, core, pair]
    for s, c in enumerate(structure):
        for core in range(n_cores):
            bins.append([c, core, s])
    placement = {}
    for gid, total in order:
        done = 0
        while done < total:
            rem = total - done
            if not bins:
                return None
            bins.sort(key=lambda b: b[0])
            if rem >= bins[-1][0]:
                cap, core, s = bins.pop()
            else:
                i = next(i for i, b in enumerate(bins) if b[0] >= rem)
                cap, core, s = bins.pop(i)
            take = min(cap, rem)
            placement[(core, s)] = (gid, done, take)
            done += take
    return placement


def _schedule(valid_lens):
    """Choose a pair structure [C_1..C_P] (identical on every core) and an
    assignment of (batch, query-half) chunk segments to (core, pair).
    Pair p expands to slots 2p/2p+1 sharing one K/V block.  Cost model:
    every core executes the full grid, so minimize total chunks first,
    then the number of pairs (each slot boundary costs drain work)."""
    import random

    nk = [max(1, -(-int(v) // CH)) for v in valid_lens]
    groups = []  # gid -> (b, qh, nchunks)
    for b in range(B):
        for qh in range(L // WH):
            groups.append((b, qh, nk[b]))
    sizes = [(gid, g[2]) for gid, g in enumerate(groups)]
    t_all = sum(s for _, s in sizes)
    tpc0 = max(1, -(-t_all // N_CORES))
    rng = random.Random(0)

    def partitions(n, max_parts):
        def rec(n, maxval, parts):
            if n == 0:
                yield list(parts)
                return
            if len(parts) == max_parts:
                return
            for v in range(min(n, maxval), 0, -1):
                parts.append(v)
                yield from rec(n - v, v, parts)
                parts.pop()

        yield from rec(n, n, [])

    # cost = total half-chunks (every core runs them) + boundary drain cost
    best = None  # (cost, structure, placement)
    for tpc in range(tpc0, tpc0 + max(nk) + 3):
        if best is not None and 2 * tpc * 429 > best[0]:
            break
        for maxp in (3, 4, 5):
            for structure in partitions(tpc, maxp):
                orders = [sorted(sizes, key=lambda x: -x[1])]
                for _ in range(300):
                    o = sizes[:]
                    rng.shuffle(o)
                    orders.append(o)
                for order in orders:
                    placement = _try_pack(groups, structure, order, N_CORES)
                    if placement is not None:
                        cost = 2 * tpc * 429 + sum(
                            max(0, 1450 - 170 * c) for c in structure
                            for _ in range(2))
                        if best is None or cost < best[0]:
                            best = (cost, structure, placement)
                        break
    assert best is not None
    _, structure, placement = best
    passign = [[None] * len(structure) for _ in range(N_CORES)]
    for (core, s), (gid, start, n) in placement.items():
        b, qh, _ = groups[gid]
        passign[core][s] = (b, qh, start, n)
    # ascending pair size: small pairs first (their input lands first, the
    # pipeline starts early), big pairs last (their inputs have time to
    # arrive while earlier slots compute).
    order = sorted(range(len(structure)), key=lambda s: structure[s])
    structure = [structure[s] for s in order]
    passign = [[row[s] for s in order] for row in passign]
    return structure, passign


# ------------------------------------------------------------- device program

def _pair_layout(structure):
    """Per-pair combined input layout: [qtE | qtO | kt | vx] in one bf16
    buffer.  Returns (offsets, total_width): offsets[p] = (qt_base, kt_off,
    vx_off)."""
    offsets = []
    base = 0
    for C in structure:
        offsets.append((base, base + WH, base + WH + C * CH))
        base += WH + C * (CH + DV)
    return offsets, base


def _build_program(structure):
    P = len(structure)           # pairs
    S = 2 * P                    # slots
    offsets, totw = _pair_layout(structure)
    slot_g0 = []                 # pt base chunk index per slot
    acc = 0
    for C in structure:
        slot_g0.extend([acc, acc + C])
        acc += 2 * C
    T = acc                      # total half-chunks
    nc = bacc.Bacc("TRN2", target_bir_lowering=False, debug=False)
    data_d = nc.dram_tensor("data", [128, totw], mybir.dt.bfloat16,
                            kind="ExternalInput").ap()
    out_d = nc.dram_tensor("out", [S * 128, OW], mybir.dt.bfloat16,
                           kind="ExternalOutput").ap()

    with tile.TileContext(nc) as tc, ExitStack() as ctx:
        sb_pool = ctx.enter_context(tc.tile_pool(name="sb", bufs=1))
        st_pool = ctx.enter_context(tc.tile_pool(name="st", bufs=2,
                                                 space="PSUM"))
        pv_pool = ctx.enter_context(tc.tile_pool(name="pv", bufs=1,
                                                 space="PSUM"))
        stage_pool = ctx.enter_context(tc.tile_pool(name="stage", bufs=2))

        data_sb = sb_pool.tile([128, totw], mybir.dt.bfloat16)
        pt_sb = sb_pool.tile([128, T * WQ], mybir.dt.bfloat16)

        # gpsimd queue: warmup memset + Schraudolph multiplier const first
        # (anything queued before the memset would delay the PE warmup chain)
        warm_sb = sb_pool.tile([128, 512], mybir.dt.bfloat16)
        nc.gpsimd.memset(warm_sb[:], 0.0)
        cmul_sb = sb_pool.tile([128, 1], mybir.dt.float32)
        nc.gpsimd.memset(cmul_sb[:], CMUL)

        # input DMAs all on the sync queue, in consumption order (a single
        # queue delivers in order; multiple queues round-robin on the wire
        # and starve the early slots).  Big pairs split qt+kt / vx.
        for p, C in enumerate(structure):
            base, kt0, vx0 = offsets[p]
            end = vx0 + C * DV
            if p == 0:
                h0 = kt0 + min(C, 2) * CH
                nc.sync.dma_start(data_sb[:, base:h0], data_d[:, base:h0])
                nc.sync.dma_start(data_sb[:, h0:end], data_d[:, h0:end])
            elif C <= 2:
                nc.sync.dma_start(data_sb[:, base:end], data_d[:, base:end])
            else:
                nc.sync.dma_start(data_sb[:, base:vx0], data_d[:, base:vx0])
                nc.sync.dma_start(data_sb[:, vx0:end], data_d[:, vx0:end])

        # PE warmup: dummy matmuls during the initial DMA wait ramp the PE
        # clock toward 2.4 GHz before real work.
        warm_ps = st_pool.tile([128, WH], mybir.dt.float32, tag="st")
        for _ in range(7):
            nc.tensor.matmul(warm_ps[:, 0:512], warm_sb[:, 0:128],
                             warm_sb[:])

        # pv accumulators: bank A per slot parity (j0-2 packed), bank B
        # shared (j3-even at cols 0:DV, j3-odd at DV:2*DV); groups
        # accumulate start=False onto pre-zeroed ranges (matmul start=True
        # resets a whole PSUM bank, so packed groups cannot use it).
        pv_a0 = pv_pool.tile([128, 512], mybir.dt.float32)
        pv_a1 = pv_pool.tile([128, 512], mybir.dt.float32)
        pv_b = pv_pool.tile([128, 512], mybir.dt.float32)

        def pv_ranges(s):
            a = pv_a0 if s % 2 == 0 else pv_a1
            b0 = (s % 2) * DV
            return a[:, 0:3 * DV], pv_b[:, b0:b0 + DV]

        for s0 in range(min(2, S)):
            pa0, pb0 = pv_ranges(s0)
            nc.vector.memset(pa0, 0.0)
            nc.vector.memset(pb0, 0.0)

        # exp units: (slot, c0, nchunks) with nchunks in {1, 2}
        units = []
        for s in range(S):
            C = structure[s // 2]
            c = 0
            while c < C:
                n = 2 if c + 1 < C else 1
                units.append((s, c, n))
                c += n

        pending = deque()
        out_q = [nc.sync, nc.gpsimd]

        def emit_front(s, c0, n):
            p = s // 2
            base, kt0, vx0 = offsets[p]
            qt = data_sb[:, base + (s % 2) * WQ:base + (s % 2) * WQ + WQ]
            st = st_pool.tile([128, WH], mybir.dt.float32, tag="st")
            for i in range(n):
                kt = data_sb[:, kt0 + (c0 + i) * CH:kt0 + (c0 + i + 1) * CH]
                nc.tensor.matmul(st[:, i * WQ:(i + 1) * WQ], kt, qt)
            w = n * WQ
            ns = NS2 if n == 2 else NS1
            p0 = (slot_g0[s] + c0) * WQ
            nc.scalar.activation(pt_sb[:, p0:p0 + ns], st[:, 0:ns],
                                 mybir.ActivationFunctionType.Exp,
                                 bias=0.0, scale=SCALE)
            if w > ns:
                cm = cmul_sb[:, 0:1]
                cbc = AP(cm.tensor, cm.offset,
                         [[cm.ap[0][0], 128], [0, w - ns]])
                nc.vector.scalar_tensor_tensor(
                    pt_sb[:, p0 + ns:p0 + w].bitcast(mybir.dt.int16),
                    st[:, ns:w], DADD, cbc,
                    mybir.AluOpType.add, mybir.AluOpType.mult)

        def emit_back(s, c0, n):
            p = s // 2
            base, kt0, vx0 = offsets[p]
            pa, pb = pv_ranges(s)
            C = structure[p]
            for i in range(n):
                c = c0 + i
                p0 = (slot_g0[s] + c) * WQ
                vx = data_sb[:, vx0 + c * DV:vx0 + (c + 1) * DV]
                for j in range(QT_N):
                    pv = pa[:, j * DV:(j + 1) * DV] if j < 3 else pb
                    nc.tensor.matmul(
                        pv, pt_sb[:, p0 + j * 128:p0 + (j + 1) * 128],
                        vx, start=False, stop=(c == C - 1),
                        skip_group_check=True)
            if c0 + n == C:
                stage = stage_pool.tile([128, OW], mybir.dt.bfloat16)
                nc.vector.tensor_copy(stage[:, 0:3 * DV], pa)
                nc.vector.tensor_copy(stage[:, 3 * DV:OW], pb)
                out_q[s % 2].dma_start(out_d[s * 128:(s + 1) * 128, :],
                                       stage[:])
                if s + 2 < S:
                    # re-zero for the slot that reuses these banks; emitted
                    # after the copies so accumulate -> copy -> zero ->
                    # accumulate is the program order
                    na, nb = pv_ranges(s + 2)
                    nc.vector.memset(na, 0.0)
                    nc.vector.memset(nb, 0.0)

        for u in units:
            emit_front(*u)
            pending.append(u)
            if len(pending) > 2:
                emit_back(*pending.popleft())
        while pending:
            emit_back(*pending.popleft())
    nc.compile()
    return nc


# ------------------------------------------------------------------- kernel

def _prep_inputs(queries, keys, values, valid_lens, structure, passign):
    offsets, totw = _pair_layout(structure)
    karange = np.arange(CH)
    in_maps = []
    for core in range(N_CORES):
        data = np.zeros((128, totw), dtype=BF16)
        for p, C in enumerate(structure):
            seg = passign[core][p]
            if seg is None:
                continue
            b, qh, cstart, ncr = seg
            base, kt0, vx0 = offsets[p]
            data[:, base:base + WH] = queries[b, qh * WH:(qh + 1) * WH, :].T
            for ci in range(ncr):
                k0 = (cstart + ci) * CH
                valid = (k0 + karange) < int(valid_lens[b])
                data[:, kt0 + ci * CH:kt0 + (ci + 1) * CH] = \
                    keys[b, k0:k0 + CH, :].T
                data[:, vx0 + ci * DV:vx0 + ci * DV + D] = \
                    values[b, k0:k0 + CH, :] * valid[:, None]
                data[:, vx0 + ci * DV + D] = valid
        in_maps.append({"data": data})
    return in_maps


def _gather(results, structure, passign):
    S = 2 * len(structure)
    num = np.zeros((B, L, D), dtype=np.float64)
    den = np.zeros((B, L), dtype=np.float64)
    for core in range(N_CORES):
        out = np.asarray(results[core]["out"], dtype=np.float64)
        out = out.reshape(S, 128, OW)
        for p in range(len(structure)):
            seg = passign[core][p]
            if seg is None:
                continue
            b, qh, _, _ = seg
            for half in range(2):
                s = 2 * p + half
                for j in range(QT_N):
                    q0 = qh * WH + half * WQ + j * 128
                    rows = slice(q0, q0 + 128)
                    num[b, rows, :] += out[s, :, j * DV:j * DV + D]
                    den[b, rows] += out[s, :, j * DV + D]
    return (num / den[:, :, None]).astype(np.float32)


def kernel(queries, keys, values, valid_lens):
    queries = np.asarray(queries, dtype=np.float32)
    keys = np.asarray(keys, dtype=np.float32)
    values = np.asarray(values, dtype=np.float32)
    valid_lens = np.asarray(valid_lens, dtype=np.int32)

    structure, passign = _schedule(valid_lens)
    nc = _build_program(structure)
    in_maps = _prep_inputs(queries, keys, values, valid_lens, structure,
                           passign)
    res = run_bass_kernel_spmd(nc, in_maps, core_ids=list(range(N_CORES)))
    return _gather(res.results, structure, passign)
